# revision 1
# baseline (speedup 1.0000x reference)
"""Trainium2 Bass kernel for nn_AdjointManifoldBlock.

Reference computes 10 RK4 steps of:
    dx/dt = v ; dv/dt = -gamma,  gamma = ((v@Wa)*(v@Wb)*tanh(x@Wx)) @ Wc

Key restructuring: the dynamics are linear in (x, v) except the rank-space
(64-dim) elementwise product. Tracking per-token rank-space state
    a = v@Wa, b = v@Wb, h = x@Wx, w = (dt/2) * (v@Wx)
every RK4 stage update is a [64,64] GEMM with composite matrices
    Caa = Wc@Wa, Cab = Wc@Wb, Cax = Wc@Wx
and the DIM-space state is only touched at the very end:
    x_T = x0 + T*v0 - (dt^2/6) * Q @ Wc        (T = STEPS*dt = 1.0)
    v_T = v0 -        (dt/6)  * S @ Wc
with S = sum_n S_n, Q = sum_n [(STEPS-1-n) S_n + P_n],
S_n = c1+2c2+2c3+c4, P_n = c1+c2+c3 (RK4 stage coefficients of step n).

Per-step recurrences (c_s = a_s*b_s*tanh(h_s)):
    a2 = a1 - (dt/2) c1@Caa ; a3 = a1 - (dt/2) c2@Caa ; a4 = a1 - dt c3@Caa
    a1' = a1 - (dt/6) S_n@Caa                       (same shape for b)
    h2 = h1 + w ; h3 = h1 + w - (dt^2/4) c1@Cax
    h4 = h1 + 2w - (dt^2/2) c2@Cax
    h1' = h1 + 2w - (dt^2/6) P_n@Cax ; w' = w - (dt^2/12) S_n@Cax

Mapping to the NeuronCore (per core: 1024 tokens, data-parallel over 8):
- two 512-token tiles are partition-stacked: rank-space tensors are
  [128, 512] (tile A ranks on partitions 0:64, tile B on 64:128)
- rank GEMMs are single full-array fp32r matmuls with block-diagonal
  [[sC,0],[0,sC]] weights (K=128 covers both tiles at once)
- a and h live in PSUM banks updated purely by PE accumulation; "+w" uses
  a block-diagonal identity matmul; S and Q accumulate in PSUM via
  scaled-identity matmuls; b-deltas and w-deltas land in scratch banks
- per stage: m = b*tanh(h) runs early on GPSIMD; c = a*m on DVE is the
  only op on the serial stage chain
- the step loop is column-split into NSPLIT independent chains (shared
  PSUM banks, disjoint column ranges) to hide cross-engine latency
- entry: PE-transposes x,v into feature-major chunks feeding DIM->RANK
  GEMMs; exit: rank-space accumulators (as stationary operands, sliced
  by token block) x Wc produce token-major output directly; x0+v0 is
  pre-added in place on GPSIMD during the steps.
"""

import json
import numpy as np

DIM = 1024
RANK = 64
STEPS = 10
DT = 0.1
BATCH, SEQ = 4, 2048
NCORES = 8
TPC = (BATCH * SEQ) // NCORES  # tokens per core = 1024
N = TPC // 2  # tokens per stacked half = 512
NCH = DIM // 128  # feature chunks = 8
NSPLIT = 2  # independent step-loop chains (column split); fp32r needs N/NSPLIT>=256

D2 = DT * DT

CAA_SCALES = [-DT / 2, DT / 2, -DT, -DT / 6, -DT / 3, 2 * DT / 3]
CAB_SCALES = [-DT / 2, -DT, -DT / 6, -DT / 3]
CAX_SCALES = [-D2 / 4, D2 / 4, -D2 / 2, D2 / 3, -D2 / 6, -D2 / 12]
IBD_SCALES = sorted(
    {1.0, 2.0}
    | {float(10 - n) for n in range(STEPS)}
    | {float(19 - 2 * n) for n in range(STEPS)}
    | {float(9 - n) for n in range(STEPS) if 9 - n > 0}
)
NV = len(CAA_SCALES) + len(CAB_SCALES) + len(CAX_SCALES) + len(IBD_SCALES)


def _vidx(kind, scale):
    if kind == "caa":
        return CAA_SCALES.index(scale)
    if kind == "cab":
        return len(CAA_SCALES) + CAB_SCALES.index(scale)
    if kind == "cax":
        return len(CAA_SCALES) + len(CAB_SCALES) + CAX_SCALES.index(scale)
    if kind == "ibd":
        return (
            len(CAA_SCALES)
            + len(CAB_SCALES)
            + len(CAX_SCALES)
            + IBD_SCALES.index(float(scale))
        )
    raise KeyError(kind)


# ---------------------------------------------------------------- host consts


def _host_consts(Wa, Wb, Wx, Wc):
    Wa64 = np.asarray(Wa, np.float64)
    Wb64 = np.asarray(Wb, np.float64)
    Wx64 = np.asarray(Wx, np.float64)
    Wc64 = np.asarray(Wc, np.float64)

    Caa = Wc64 @ Wa64  # [64, 64]; row index = coeff rank (contraction side)
    Cab = Wc64 @ Wb64
    Cax = Wc64 @ Wx64
    I64 = np.eye(RANK)

    cmp_mats = (
        [Caa * sc for sc in CAA_SCALES]
        + [Cab * sc for sc in CAB_SCALES]
        + [Cax * sc for sc in CAX_SCALES]
        + [I64 * sc for sc in IBD_SCALES]
    )
    bdarr = np.stack(cmp_mats).astype(np.float32)  # [NV, 64, 64]
    bdarr = np.ascontiguousarray(bdarr.transpose(1, 0, 2))  # [64, NV, 64]

    # start weights: tensor t in (Wa, Wb, Wx, (dt/2)Wx), chunk k in 0..7
    stk = np.stack(
        [W.reshape(NCH, 128, RANK) for W in (Wa64, Wb64, Wx64, (DT / 2) * Wx64)]
    )  # [4, 8, 128, 64]
    wsa = np.ascontiguousarray(
        stk.transpose(2, 0, 1, 3).reshape(128, 4 * NCH, RANK)
    ).astype(np.float32)  # [128, 32, 64] (A-tile weights, natural)
    # B-tile weights are the same data placed in columns 64:128 of a
    # zeroed [128, 32, 128]; ship only the data half, zero-fill on device.
    wcv = np.asarray(-(DT / 6) * Wc64, np.float32)  # [64, 1024]
    wcx = np.asarray(-(D2 / 6) * Wc64, np.float32)
    ident = np.eye(128, dtype=np.float32)

    return {"bd": bdarr, "wsa": wsa, "wcv": wcv, "wcx": wcx, "ident": ident}


# ----------------------------------------------------------- BIR wait postpass


def _split_waits(data: bytes) -> bytes:
    """This walrus build accepts only one inline sync wait per instruction;
    move excess waits onto NoOps inserted before the instruction (the
    engine sequencer processes them in order, so semantics are identical)."""
    bir = json.loads(data)
    for fn in bir["functions"]:
        for blk in fn["blocks"]:
            out = []
            k = 0
            for inst in blk["instructions"]:
                si = inst.get("sync_info")
                if si and len(si.get("on_wait", [])) > 1:
                    waits = si["on_wait"]
                    pre = []
                    while len(waits) > 1:
                        chunk, waits = waits[:1], waits[1:]
                        k += 1
                        pre.append(
                            {
                                "name": f"{inst['name']}-w{k}",
                                "opcode": "NoOp",
                                "engine": inst["engine"],
                                "ins": [],
                                "outs": [],
                                "sync_info": {"on_wait": chunk, "on_update": []},
                            }
                        )
                    si["on_wait"] = waits
                    out.extend(pre)
                out.append(inst)
            blk["instructions"] = out
    return json.dumps(bir).encode()


# ---------------------------------------------------------------- bass builder

_NC_CACHE = None
DEBUG = False
DEBUG_STEP = 0


def _build_bass():
    global _NC_CACHE
    if _NC_CACHE is not None:
        return _NC_CACHE

    import concourse.bass as bass
    import concourse.tile as tile
    import concourse.mybir as mybir

    F32 = mybir.dt.float32
    F32R = mybir.dt.float32r
    TANH = mybir.ActivationFunctionType.Tanh
    COPY = mybir.ActivationFunctionType.Copy

    nc = bass.Bass("TRN2", target_bir_lowering=False, debug=False, num_devices=1)

    xin = nc.dram_tensor("xin", [TPC, DIM], F32, kind="ExternalInput").ap()
    vin = nc.dram_tensor("vin", [TPC, DIM], F32, kind="ExternalInput").ap()
    bdm = nc.dram_tensor("bd", [64, NV, 64], F32, kind="ExternalInput").ap()
    wsa = nc.dram_tensor("wsa", [128, 4 * NCH, RANK], F32, kind="ExternalInput").ap()
    wcv = nc.dram_tensor("wcv", [RANK, DIM], F32, kind="ExternalInput").ap()
    wcx = nc.dram_tensor("wcx", [RANK, DIM], F32, kind="ExternalInput").ap()
    idn = nc.dram_tensor("ident", [128, 128], F32, kind="ExternalInput").ap()
    xout = nc.dram_tensor("xout", [TPC, DIM], F32, kind="ExternalOutput").ap()
    vout = nc.dram_tensor("vout", [TPC, DIM], F32, kind="ExternalOutput").ap()
    dbg = {}
    if DEBUG:
        for nm in ("a1", "h1", "b1", "w", "wd", "c1", "c2", "c3", "c4", "S", "Q"):
            dbg[nm] = nc.dram_tensor(
                f"dbg_{nm}", [128, N], F32, kind="ExternalOutput"
            ).ap()

    NC2 = N // NSPLIT  # columns per chain

    with tile.TileContext(nc) as tc:
        with (
            tc.tile_pool(name="consts", bufs=1) as consts,
            tc.tile_pool(name="work", bufs=1) as work,
            tc.tile_pool(name="bpool", bufs=3) as bpool,
            tc.tile_pool(name="cpool", bufs=6) as cpool,
            tc.tile_pool(name="tpool", bufs=4) as tpool,
            tc.tile_pool(name="mpool", bufs=4) as mpool,
            tc.tile_pool(name="ps_main", bufs=1, space="PSUM") as ps_main,
        ):
            # ---------------- input loads (v first: its transposes and
            # GEMMs are the front of the pipeline)
            s_xtok = consts.tile([128, NCH, DIM], F32, tag="xtok")
            s_vtok = consts.tile([128, NCH, DIM], F32, tag="vtok")

            # persistent PSUM accumulators; memset clears both values and
            # makes any stale has_written state harmless, so every matmul
            # below can use start=False in any order.
            # chain-private a and h banks: avoids the PSUM same-bank
            # PE-write / engine-read serialization between the two chains.
            # cols 0:NC2 of Ba hold a; cols NC2:2NC2 hold the cumulative
            # w-delta (w_n = w0 + wdcum), which needs no per-step clearing.
            p_a = ps_main.tile([128, N], F32, tag="Ba")
            p_a2 = ps_main.tile([128, N], F32, tag="Ba2")
            p_h = ps_main.tile([128, NC2], F32, tag="Bh")
            p_h2 = ps_main.tile([128, NC2], F32, tag="Bh2")
            p_b0 = ps_main.tile([128, N], F32, tag="BS")
            p_w0 = ps_main.tile([128, N], F32, tag="BQ")
            for p in (p_a, p_a2, p_h, p_h2, p_b0, p_w0):
                nc.vector.memset(p[:], 0.0)
            p_a_ch = [p_a, p_a2]
            p_h_ch = [p_h, p_h2]

            def bdw(kind, scale):
                return s_bd[:, _vidx(kind, scale), :].bitcast(F32R)

            # ---------------- entry: transposes + DIM->RANK GEMMs
            with (
                tc.tile_pool(name="entry", bufs=1) as entry,
                tc.tile_pool(name="stream", bufs=2) as stream,
                tc.tile_pool(name="ps_tr", bufs=2, space="PSUM") as ps_tr,
            ):
                s_id = entry.tile([128, 128], F32, tag="ident")
                nc.sync.dma_start(s_id[:].bitcast(F32R), idn[:].bitcast(F32R))
                s_wsa = entry.tile([128, 4 * NCH, RANK], F32, tag="wsa")
                nc.sync.dma_start(s_wsa[:].bitcast(F32R), wsa[:].bitcast(F32R))
                # A-half token blocks (0-3) of both tensors first: the
                # A-half transpose+GEMM pipeline starts while B still loads
                s_wsb = entry.tile([128, 4 * NCH, 128], F32, tag="wsb")
                nc.vector.memset(s_wsb[:], 0.0)
                for tb in range(NCH // 2):
                    nc.sync.dma_start(
                        s_vtok[:, tb, :].bitcast(F32R),
                        vin[tb * 128 : (tb + 1) * 128, :].bitcast(F32R),
                    )
                for tb in range(NCH // 2):
                    nc.sync.dma_start(
                        s_xtok[:, tb, :].bitcast(F32R),
                        xin[tb * 128 : (tb + 1) * 128, :].bitcast(F32R),
                    )
                # B-tile padded weights: zero-fill + one strided DMA of the
                # data half into columns 64:128
                nc.sync.dma_start(
                    s_wsb[:, :, 64:128].bitcast(F32R), wsa[:].bitcast(F32R)
                )
                for tb in range(NCH // 2, NCH):
                    nc.sync.dma_start(
                        s_vtok[:, tb, :].bitcast(F32R),
                        vin[tb * 128 : (tb + 1) * 128, :].bitcast(F32R),
                    )
                for tb in range(NCH // 2, NCH):
                    nc.sync.dma_start(
                        s_xtok[:, tb, :].bitcast(F32R),
                        xin[tb * 128 : (tb + 1) * 128, :].bitcast(F32R),
                    )

                # late consts (not needed until steps / exit); block-diag
                # [NV][128,128] built from compact [NV][64,64]: zero-fill,
                # then two strided DMAs into the diagonal blocks
                s_bd = consts.tile([128, NV, 128], F32, tag="bd")
                nc.vector.memset(s_bd[:], 0.0)
                nc.sync.dma_start(
                    s_bd[0:64, :, 0:64].bitcast(F32R), bdm[:].bitcast(F32R)
                )
                nc.sync.dma_start(
                    s_bd[64:128, :, 64:128].bitcast(F32R), bdm[:].bitcast(F32R)
                )
                s_wcv = consts.tile([128, DIM], F32, tag="wcv")
                nc.sync.dma_start(
                    s_wcv[0:64, :].bitcast(F32R), wcv[:].bitcast(F32R)
                )
                nc.sync.dma_start(
                    s_wcv[64:128, :].bitcast(F32R), wcv[:].bitcast(F32R)
                )
                s_wcx = consts.tile([128, DIM], F32, tag="wcx")
                nc.sync.dma_start(
                    s_wcx[0:64, :].bitcast(F32R), wcx[:].bitcast(F32R)
                )
                nc.sync.dma_start(
                    s_wcx[64:128, :].bitcast(F32R), wcx[:].bitcast(F32R)
                )

                for half in range(2):
                    for k in range(NCH):
                        vT = stream.tile([128, N], F32, tag="vT")
                        xT = stream.tile([128, N], F32, tag="xT")
                        for src_tok, dst in ((s_vtok, vT), (s_xtok, xT)):
                            p_tr = ps_tr.tile([128, N], F32R, tag="tr")
                            for q in range(4):
                                tb = half * 4 + q
                                nc.tensor.transpose(
                                    p_tr[:, q * 128 : (q + 1) * 128],
                                    src_tok[
                                        :, tb, k * 128 : (k + 1) * 128
                                    ].bitcast(F32R),
                                    s_id[:].bitcast(F32R),
                                )
                            nc.scalar.activation(
                                dst[:].bitcast(F32R),
                                p_tr[:].bitcast(F32),
                                COPY,
                            )
                        # MMs consuming this half only (A-half: natural
                        # weights, M=64; B-half: zero-padded, M=128)
                        for bank, tsel, src in (
                            (p_a, 0, vT),
                            (p_b0, 1, vT),
                            (p_h, 2, xT),
                            (p_w0, 3, vT),
                        ):
                            banks = (
                                (p_a_ch if tsel == 0 else p_h_ch)
                                if tsel in (0, 2)
                                else [bank]
                            )
                            for ci, bk in enumerate(banks):
                                split = tsel in (0, 2)
                                lo = ci * NC2 if split else 0
                                cw = NC2 if split else N
                                osl = slice(0, cw)
                                if half == 0:
                                    nc.tensor.matmul(
                                        bk[0:64, osl],
                                        s_wsa[:, tsel * NCH + k, :].bitcast(
                                            F32R
                                        ),
                                        src[:, lo : lo + cw].bitcast(F32R),
                                        start=False,
                                        stop=False,
                                        skip_group_check=True,
                                    )
                                else:
                                    nc.tensor.matmul(
                                        bk[:, osl],
                                        s_wsb[:, tsel * NCH + k, :].bitcast(
                                            F32R
                                        ),
                                        src[:, lo : lo + cw].bitcast(F32R),
                                        start=False,
                                        stop=(k == NCH - 1),
                                        skip_group_check=True,
                                    )

            # b and w to SBUF (per chain); banks become the S/Q accumulators
            chains = []
            for ch in range(NSPLIT):
                sl = slice(ch * NC2, (ch + 1) * NC2)
                b1 = bpool.tile([128, NC2], F32, tag=f"b1_{ch}")
                nc.vector.tensor_copy(b1[:], p_b0[:, sl])
                w0 = bpool.tile([128, NC2], F32, tag=f"w0_{ch}")
                nc.vector.tensor_copy(w0[:].bitcast(F32R), p_w0[:, sl])
                chains.append({"b1": b1, "w0": w0, "w": w0, "sl": sl})
            p_S = ps_main.tile([128, N], F32, tag="BS")
            p_Q = ps_main.tile([128, N], F32, tag="BQ")

            # x0 += v0 happens in place on GPSIMD, spread across the steps

            # ---------------- the 10 RK4 steps, fully unrolled
            with tc.tile_pool(name="ps_step", bufs=1, space="PSUM") as ps_step:

                def mm(bank, sl, kind, scale, rhs_view, start=False, stop=False):
                    nc.tensor.matmul(
                        bank[:, sl],
                        bdw(kind, scale),
                        rhs_view,
                        start=start,
                        stop=stop,
                        skip_group_check=True,
                    )

                def step_chain(n, st, db_tiles):
                    """Emit one RK4 step for one chain; yields between stages
                    so chains can be interleaved."""
                    ch = st["ch"]
                    sl = st["sl"]
                    pa = p_a_ch[ch]
                    ph = p_h_ch[ch]
                    asl = slice(0, NC2)
                    wsl = slice(NC2, 2 * NC2)
                    last = n == STEPS - 1
                    q1, q23, q4 = float(10 - n), float(19 - 2 * n), float(9 - n)

                    def tanh(idx):
                        t = tpool.tile([128, NC2], F32, tag=f"tanh_{ch}")
                        nc.scalar.activation(t[:], ph[:, asl], TANH)
                        return t

                    def premul(b_s, t_s, stage):
                        # m = b * tanh(h): off the DVE, on GPSIMD
                        m = mpool.tile([128, NC2], F32, tag=f"m_{ch}")
                        nc.gpsimd.tensor_mul(m[:], b_s[:], t_s[:])
                        return m

                    def coeff(m_s):
                        # c = a * m: PSUM x SBUF on DVE, f32r out
                        c = cpool.tile([128, NC2], F32, tag=f"c_{ch}")
                        nc.vector.tensor_mul(c[:].bitcast(F32R), pa[:, asl], m_s[:])
                        if DEBUG and n == DEBUG_STEP:
                            st.setdefault("cdump", []).append(c)
                        return c[:].bitcast(F32R)

                    def badd(db):
                        b = bpool.tile([128, NC2], F32, tag=f"bs_{ch}")
                        nc.vector.tensor_add(b[:], st["b1"][:], db[:])
                        return b

                    # stage 1
                    t1 = tanh(1)
                    mm(ph, asl, "ibd", 1.0, st["w"][:].bitcast(F32R), stop=True)
                    t2 = tanh(2)
                    m1 = premul(st["b1"], t1, 1)
                    c1 = coeff(m1)
                    # start=True clears has_written for the WHOLE bank, so
                    # only the very first write (chain 0) may carry it
                    mm(p_S, sl, "ibd", 1.0, c1, start=(n == 0 and ch == 0))
                    mm(p_Q, sl, "ibd", q1, c1, start=(n == 0 and ch == 0))
                    mm(pa, asl, "caa", -DT / 2, c1, stop=True)  # a2
                    mm(db_tiles[0], slice(0, NC2), "cab", -DT / 2, c1, start=True, stop=True)
                    b2 = badd(db_tiles[0])
                    mm(ph, asl, "cax", -D2 / 4, c1, stop=True)  # h3
                    t3 = tanh(3)
                    if not last:
                        mm(pa, wsl, "cax", -D2 / 12, c1)
                    yield

                    # stage 2
                    m2 = premul(b2, t2, 2)
                    c2 = coeff(m2)
                    mm(p_S, sl, "ibd", 2.0, c2)
                    mm(p_Q, sl, "ibd", q23, c2)
                    mm(pa, asl, "caa", DT / 2, c1)
                    mm(pa, asl, "caa", -DT / 2, c2, stop=True)  # a3
                    mm(db_tiles[1], slice(0, NC2), "cab", -DT / 2, c2, start=True, stop=True)
                    b3 = badd(db_tiles[1])
                    mm(ph, asl, "ibd", 1.0, st["w"][:].bitcast(F32R))
                    mm(ph, asl, "cax", D2 / 4, c1)
                    mm(ph, asl, "cax", -D2 / 2, c2, stop=True)  # h4
                    t4 = tanh(4)
                    if not last:
                        mm(pa, wsl, "cax", -D2 / 6, c2)
                    yield

                    # stage 3
                    m3 = premul(b3, t3, 3)
                    c3 = coeff(m3)
                    mm(p_S, sl, "ibd", 2.0, c3)
                    mm(p_Q, sl, "ibd", q23, c3, stop=last)
                    mm(pa, asl, "caa", DT / 2, c2)
                    mm(pa, asl, "caa", -DT, c3, stop=True)  # a4
                    mm(db_tiles[2], slice(0, NC2), "cab", -DT, c3, start=True, stop=True)
                    b4 = badd(db_tiles[2])
                    if not last:
                        mm(pa, wsl, "cax", -D2 / 6, c3)
                    yield

                    # stage 4
                    m4 = premul(b4, t4, 4)
                    c4 = coeff(m4)
                    mm(p_S, sl, "ibd", 1.0, c4, stop=last)
                    if not last:
                        mm(p_Q, sl, "ibd", q4, c4)
                        mm(pa, asl, "caa", 2 * DT / 3, c3)
                        mm(pa, asl, "caa", -DT / 6, c1)
                        mm(pa, asl, "caa", -DT / 3, c2)
                        mm(pa, asl, "caa", -DT / 6, c4, stop=True)  # a1'
                        mm(ph, asl, "cax", D2 / 3, c2)
                        mm(ph, asl, "cax", -D2 / 6, c1)
                        mm(ph, asl, "cax", -D2 / 6, c3, stop=True)  # h1'
                        mm(db_tiles[3], slice(0, NC2), "cab", -DT / 6, c1, start=True)
                        mm(db_tiles[3], slice(0, NC2), "cab", -DT / 3, c2)
                        mm(db_tiles[3], slice(0, NC2), "cab", -DT / 3, c3)
                        mm(db_tiles[3], slice(0, NC2), "cab", -DT / 6, c4, stop=True)
                        nb1 = bpool.tile([128, NC2], F32, tag=f"b1_{st['ch']}")
                        nc.vector.tensor_add(nb1[:], st["b1"][:], db_tiles[3][:])
                        st["b1"] = nb1
                        mm(pa, wsl, "cax", -D2 / 12, c4, stop=True)
                        nw = bpool.tile([128, NC2], F32, tag=f"w_{st['ch']}")
                        nc.vector.tensor_add(
                            nw[:].bitcast(F32R), st["w0"][:], pa[:, wsl]
                        )
                        st["w"] = nw
                    yield

                for ch, st in enumerate(chains):
                    st["ch"] = ch
                for n in range(STEPS):
                    # shared scratch banks for this step; both chains use
                    # disjoint column halves (memset once per allocation
                    # round: values are fully written by their MMs after a
                    # region memset clears stale has_written semantics)
                    last_step = n == STEPS - 1
                    db_per_chain = []
                    for ci in range(NSPLIT):
                        da = ps_step.tile([128, NC2], F32, tag=f"db{ci}")
                        db = ps_step.tile([128, NC2], F32, tag=f"db{ci}")
                        dc = ps_step.tile([128, NC2], F32, tag=f"db{ci}")
                        if not last_step:
                            dd = ps_step.tile([128, NC2], F32, tag=f"db{ci}")
                        else:
                            dd = None
                        db_per_chain.append([da, db, dc, dd])
                    if DEBUG and n == DEBUG_STEP:
                        for ci2, st2 in enumerate(chains):
                            csl2 = slice(ci2 * NC2, (ci2 + 1) * NC2)
                            pa2_d = p_a_ch[ci2] if NSPLIT == 2 else p_a
                            asl2 = slice(0, NC2) if NSPLIT == 2 else csl2
                            tmp_a2 = work.tile([128, NC2], F32, tag=f"dbga{ci2}")
                            nc.vector.tensor_copy(tmp_a2[:], pa2_d[:, asl2])
                            nc.sync.dma_start(dbg["a1"][:, csl2], tmp_a2[:])
                            nc.sync.dma_start(dbg["b1"][:, csl2], st2["b1"][:])
                            nc.sync.dma_start(dbg["w"][:, csl2], st2["w"][:])
                        tmp_h2 = work.tile([128, N], F32, tag="dbgh")
                        for ci3 in range(NSPLIT):
                            nc.vector.tensor_copy(
                                tmp_h2[:, ci3 * NC2 : (ci3 + 1) * NC2],
                                p_h_ch[ci3][:, 0:NC2],
                            )
                        nc.sync.dma_start(dbg["h1"][:], tmp_h2[:])
                    gens = [
                        step_chain(n, st, db_per_chain[st["ch"]])
                        for st in chains
                    ]
                    alive = True
                    while alive:
                        alive = False
                        for g in gens:
                            try:
                                next(g)
                                alive = True
                            except StopIteration:
                                pass
                    if DEBUG and n == DEBUG_STEP:
                        for ci, st in enumerate(chains):
                            csl = slice(ci * NC2, (ci + 1) * NC2)
                            for j, ct in enumerate(st.get("cdump", [])):
                                nc.sync.dma_start(
                                    dbg[f"c{j+1}"][:, csl], ct[:]
                                )


            # ---------------- exit: RANK->DIM GEMMs, token-major output
            # per-chain copies on ACT: each chain's end-GEMMs can start as
            # soon as its own columns are final, and DVE stays free for the
            # output adds
            s_S = work.tile([128, N], F32, tag="sS")
            s_Q = work.tile([128, N], F32, tag="sQ")
            for ci in range(NSPLIT):
                csl = slice(ci * NC2, (ci + 1) * NC2)
                nc.scalar.activation(s_S[:, csl].bitcast(F32R), p_S[:, csl], COPY)
                nc.scalar.activation(s_Q[:, csl].bitcast(F32R), p_Q[:, csl], COPY)
            if DEBUG:
                nc.sync.dma_start(dbg["S"][:], s_S[:])
                nc.sync.dma_start(dbg["Q"][:], s_Q[:])

            with (
                tc.tile_pool(name="ps_end", bufs=2, space="PSUM") as ps_end,
                tc.tile_pool(name="opool", bufs=4) as opool,
            ):
                for th in range(2):  # tile half (A/B)
                    for tb4 in range(4):  # token block within half
                        tb = th * 4 + tb4
                        for dh in range(2):  # dim half
                            lhs_S = s_S[
                                th * 64 : (th + 1) * 64,
                                tb4 * 128 : (tb4 + 1) * 128,
                            ].bitcast(F32R)
                            lhs_Q = s_Q[
                                th * 64 : (th + 1) * 64,
                                tb4 * 128 : (tb4 + 1) * 128,
                            ].bitcast(F32R)
                            rv = s_wcv[
                                th * 64 : (th + 1) * 64, dh * N : (dh + 1) * N
                            ].bitcast(F32R)
                            rx = s_wcx[
                                th * 64 : (th + 1) * 64, dh * N : (dh + 1) * N
                            ].bitcast(F32R)
                            # v out
                            pv = ps_end.tile([128, N], F32, tag="eo")
                            nc.tensor.matmul(
                                pv[:],
                                lhs_S,
                                rv,
                                start=True,
                                stop=True,
                                tile_position=(64 * th, 0),
                                skip_group_check=True,
                            )
                            # v-add path off DVE: ACT copies PSUM out,
                            # GPSIMD adds (DVE keeps the x-adds)
                            pvc = opool.tile([128, N], F32, tag="pvc")
                            nc.scalar.activation(pvc[:], pv[:], COPY)
                            ov = opool.tile([128, N], F32, tag="ov")
                            nc.gpsimd.tensor_add(
                                ov[:], s_vtok[:, tb, dh * N : (dh + 1) * N].bitcast(F32), pvc[:]
                            )
                            nc.sync.dma_start(
                                vout[
                                    tb * 128 : (tb + 1) * 128,
                                    dh * N : (dh + 1) * N,
                                ],
                                ov[:],
                            )
                            # x out = (x0 + v0) + Q-gemm   (x0+v0 pre-added)
                            px = ps_end.tile([128, N], F32, tag="eo")
                            nc.tensor.matmul(
                                px[:],
                                lhs_Q,
                                rx,
                                start=True,
                                stop=True,
                                tile_position=(64 * th, 0),
                                skip_group_check=True,
                            )
                            oxh = opool.tile([128, N], F32, tag="oxh")
                            nc.vector.tensor_add(
                                oxh[:],
                                s_xtok[:, tb, dh * N : (dh + 1) * N].bitcast(F32),
                                px[:],
                            )
                            ox = opool.tile([128, N], F32, tag="ox")
                            nc.vector.tensor_add(
                                ox[:],
                                oxh[:],
                                s_vtok[:, tb, dh * N : (dh + 1) * N].bitcast(F32),
                            )
                            nc.sync.dma_start(
                                xout[
                                    tb * 128 : (tb + 1) * 128,
                                    dh * N : (dh + 1) * N,
                                ],
                                ox[:],
                            )

    orig = nc.to_json_bytes
    nc.to_json_bytes = lambda: _split_waits(orig())
    _NC_CACHE = nc
    return nc


# -------------------------------------------------------------------- driver


def _run(x, v, Wa, Wb, Wx, Wc, trace=False):
    from concourse.bass_utils import run_bass_kernel_spmd

    x = np.asarray(x, np.float32).reshape(BATCH * SEQ, DIM)
    v = np.asarray(v, np.float32).reshape(BATCH * SEQ, DIM)
    consts = _host_consts(Wa, Wb, Wx, Wc)

    nc = _build_bass()
    in_maps = []
    for c in range(NCORES):
        m = {
            "xin": np.ascontiguousarray(x[c * TPC : (c + 1) * TPC]),
            "vin": np.ascontiguousarray(v[c * TPC : (c + 1) * TPC]),
        }
        m.update(consts)
        in_maps.append(m)

    res = run_bass_kernel_spmd(
        nc, in_maps, core_ids=list(range(NCORES)), trace=trace
    )
    xo = np.concatenate([res.results[c]["xout"] for c in range(NCORES)], axis=0)
    vo = np.concatenate([res.results[c]["vout"] for c in range(NCORES)], axis=0)
    return (xo.reshape(BATCH, SEQ, DIM), vo.reshape(BATCH, SEQ, DIM)), res


def kernel(x, v, Wa, Wb, Wx, Wc):
    (xo, vo), _ = _run(x, v, Wa, Wb, Wx, Wc, trace=False)
    return xo, vo



# revision 3
# speedup vs baseline: 1.1608x; 1.1608x over previous
"""Trainium2 Bass kernel for nn_AdjointManifoldBlock.

Reference computes 10 RK4 steps of:
    dx/dt = v ; dv/dt = -gamma,  gamma = ((v@Wa)*(v@Wb)*tanh(x@Wx)) @ Wc

Rank-space restructuring (per token, rank=64 state):
    a = v@Wa, b = v@Wb, h = x@Wx, w = (dt/2) v@Wx
    c_s = a_s * b_s * tanh(h_s)   per RK4 stage
    every stage update is a [64,64] GEMM with Caa=Wc@Wa, Cab=Wc@Wb, Cax=Wc@Wx
    x_T = x0 + v0 - (dt^2/6) Q @ Wc,  v_T = v0 - (dt/6) S @ Wc
    S = sum S_n, Q = sum [(9-n) S_n + P_n]

Differences vs the earlier version of this kernel:
  - inputs are shipped twice: token-major fp32 (exit adds) and host-transposed
    feature-major fp16 (entry GEMMs) -> no PE transposes, no ACT copies
  - all step-loop matmul operands are fp16 (1 cycle/row at any width)
  - the b-chain lives in PSUM like a (PE undo-accumulation), killing the
    per-stage DVE badd; both per-stage products run on DVE (GPSIMD has no
    PSUM port)
  - the w-chain is derived from the S accumulator: w_n = w0 - (dt^2/12)
    Scum_n @ Cax, tracked in PSUM via per-step delta matmuls from fp16
    snapshots of S (2 matmuls/step instead of 4)
  - exit: x0+v0 pre-added on GPSIMD (idle otherwise) during the steps;
    final adds on DVE straight from PSUM; fp32 x,v DMA'd during the steps

Layout per core (1024 tokens): partition dim = [halfA ranks 0:64 | halfB
ranks 64:128], halves = tokens 0:512 / 512:1024; NSPLIT=2 column chains
(256 cols each) interleaved stage-by-stage for cross-engine overlap.
"""

import json
import numpy as np
import ml_dtypes

DIM = 1024
RANK = 64
STEPS = 10
DT = 0.1
BATCH, SEQ = 4, 2048
NCORES = 8
TPC = (BATCH * SEQ) // NCORES  # tokens per core = 1024
NH = TPC // 2  # tokens per stacked half = 512
NCH = DIM // 128  # feature chunks = 8
NSPLIT = 2
NC2 = NH // NSPLIT  # columns per chain = 256

D2 = DT * DT

CAA_SC = [-DT / 2, DT / 2, -DT, -DT / 6, -DT / 3, 2 * DT / 3]
CAB_SC = list(CAA_SC)
CAX_SC = [-D2 / 4, D2 / 4, -D2 / 2, D2 / 3, -D2 / 6, -D2 / 12, D2 / 12]
IBD_SC = sorted(
    {1.0, 2.0}
    | {float(10 - n) for n in range(STEPS)}
    | {float(19 - 2 * n) for n in range(STEPS)}
    | {float(9 - n) for n in range(STEPS) if 9 - n > 0}
)
NV = len(CAA_SC) + len(CAB_SC) + len(CAX_SC) + len(IBD_SC)

F16NP = np.float16


def _vidx(kind, scale):
    if kind == "caa":
        return CAA_SC.index(scale)
    if kind == "cab":
        return len(CAA_SC) + CAB_SC.index(scale)
    if kind == "cax":
        return len(CAA_SC) + len(CAB_SC) + CAX_SC.index(scale)
    if kind == "ibd":
        return len(CAA_SC) + len(CAB_SC) + len(CAX_SC) + IBD_SC.index(float(scale))
    raise KeyError(kind)


# ---------------------------------------------------------------- host consts


def _host_consts(Wa, Wb, Wx, Wc):
    Wa64 = np.asarray(Wa, np.float64)
    Wb64 = np.asarray(Wb, np.float64)
    Wx64 = np.asarray(Wx, np.float64)
    Wc64 = np.asarray(Wc, np.float64)

    Caa = Wc64 @ Wa64  # [64, 64]; row index = contraction side
    Cab = Wc64 @ Wb64
    Cax = Wc64 @ Wx64
    I64 = np.eye(RANK)

    mats = (
        [Caa * s for s in CAA_SC]
        + [Cab * s for s in CAB_SC]
        + [Cax * s for s in CAX_SC]
        + [I64 * s for s in IBD_SC]
    )
    bd = np.zeros((NV, 128, 128), np.float64)
    for i, m in enumerate(mats):
        bd[i, 0:64, 0:64] = m
        bd[i, 64:128, 64:128] = m
    bd = np.ascontiguousarray(bd.transpose(1, 0, 2)).astype(F16NP)  # [128, NV, 128]

    stk = np.stack(
        [W.reshape(NCH, 128, RANK) for W in (Wa64, Wb64, Wx64, (DT / 2) * Wx64)]
    )  # [4, 8, 128, 64]
    wsa = np.ascontiguousarray(stk.transpose(2, 0, 1, 3).reshape(128, 4 * NCH, RANK)).astype(
        F16NP
    )
    wcv1 = -(DT / 6) * Wc64  # [64, 1024]
    wcx1 = -(D2 / 6) * Wc64
    wcv = np.concatenate([wcv1, wcv1], axis=0).astype(F16NP)  # [128, 1024] duplicated
    wcx = np.concatenate([wcx1, wcx1], axis=0).astype(F16NP)

    return {"bd": bd, "wsa": wsa, "wcv": wcv, "wcx": wcx}


# ----------------------------------------------------------- BIR wait postpass


def _split_waits(data: bytes) -> bytes:
    """This walrus build accepts only one inline sync wait per instruction;
    move excess waits onto NoOps inserted before the instruction (the
    engine sequencer processes them in order, so semantics are identical)."""
    bir = json.loads(data)
    for fn in bir["functions"]:
        for blk in fn["blocks"]:
            out = []
            k = 0
            for inst in blk["instructions"]:
                si = inst.get("sync_info")
                if si and len(si.get("on_wait", [])) > 1:
                    waits = si["on_wait"]
                    pre = []
                    while len(waits) > 1:
                        chunk, waits = waits[:1], waits[1:]
                        k += 1
                        pre.append(
                            {
                                "name": f"{inst['name']}-w{k}",
                                "opcode": "NoOp",
                                "engine": inst["engine"],
                                "ins": [],
                                "outs": [],
                                "sync_info": {"on_wait": chunk, "on_update": []},
                            }
                        )
                    si["on_wait"] = waits
                    out.extend(pre)
                out.append(inst)
            blk["instructions"] = out
    return json.dumps(bir).encode()


# ---------------------------------------------------------------- bass builder

_NC_CACHE = None


def _build_bass():
    global _NC_CACHE
    if _NC_CACHE is not None:
        return _NC_CACHE

    import concourse.bass as bass
    import concourse.tile as tile
    import concourse.mybir as mybir

    F32 = mybir.dt.float32
    F16 = mybir.dt.float16
    TANH = mybir.ActivationFunctionType.Tanh
    COPY = mybir.ActivationFunctionType.Copy

    nc = bass.Bass("TRN2", target_bir_lowering=False, debug=False, num_devices=1)

    xin = nc.dram_tensor("xin", [TPC, DIM], F32, kind="ExternalInput").ap()
    vin = nc.dram_tensor("vin", [TPC, DIM], F32, kind="ExternalInput").ap()
    xtr = nc.dram_tensor("xt", [DIM, TPC], F16, kind="ExternalInput").ap()
    vtr = nc.dram_tensor("vt", [DIM, TPC], F16, kind="ExternalInput").ap()
    bdm = nc.dram_tensor("bd", [128, NV, 128], F16, kind="ExternalInput").ap()
    wsa = nc.dram_tensor("wsa", [128, 4 * NCH, RANK], F16, kind="ExternalInput").ap()
    wcv = nc.dram_tensor("wcv", [128, DIM], F16, kind="ExternalInput").ap()
    wcx = nc.dram_tensor("wcx", [128, DIM], F16, kind="ExternalInput").ap()
    xout = nc.dram_tensor("xout", [TPC, DIM], F32, kind="ExternalOutput").ap()
    vout = nc.dram_tensor("vout", [TPC, DIM], F32, kind="ExternalOutput").ap()

    with tile.TileContext(nc) as tc:
        with (
            tc.tile_pool(name="consts", bufs=1) as consts,
            tc.tile_pool(name="tpool", bufs=6) as tpool,
            tc.tile_pool(name="mpool", bufs=4) as mpool,
            tc.tile_pool(name="cpool", bufs=6) as cpool,
            tc.tile_pool(name="spool", bufs=3) as spool,
            tc.tile_pool(name="epool", bufs=1) as epool,
            tc.tile_pool(name="opool", bufs=4) as opool,
            tc.tile_pool(name="ps", bufs=1, space="PSUM") as ps,
        ):
            # ---------------- tiles
            s_bd = consts.tile([128, NV, 128], F16, tag="bd")
            s_wsa = consts.tile([128, 4 * NCH, RANK], F16, tag="wsa")
            s_wcv = consts.tile([128, DIM], F16, tag="wcv")
            s_wcx = consts.tile([128, DIM], F16, tag="wcx")
            s_vt = consts.tile([128, NCH, TPC], F16, tag="vt")
            s_xt = consts.tile([128, NCH, TPC], F16, tag="xt")
            s_vtok = consts.tile([128, NCH, DIM], F32, tag="vtok")
            s_xtok = consts.tile([128, NCH, DIM], F32, tag="xtok")

            B_a = [ps.tile([128, 2 * NC2], F32, tag=f"Ba{c}", name=f"Ba{c}") for c in range(2)]
            B_b = [ps.tile([128, 2 * NC2], F32, tag=f"Bb{c}", name=f"Bb{c}") for c in range(2)]
            B_h = [ps.tile([128, 2 * NC2], F32, tag=f"Bh{c}", name=f"Bh{c}") for c in range(2)]
            B_S = ps.tile([128, NH], F32, tag="BS")
            B_Q = ps.tile([128, NH], F32, tag="BQ")

            asl = slice(0, NC2)  # a/b/h state columns within chain banks
            wsl = slice(NC2, 2 * NC2)  # w columns within B_a

            # memsets: a-banks fully (w region too); b/h state region only
            for c in range(2):
                nc.vector.memset(B_a[c][:], 0.0)
                nc.vector.memset(B_b[c][:, asl], 0.0)
                nc.vector.memset(B_h[c][:, asl], 0.0)
            nc.vector.memset(B_S[:], 0.0)
            nc.vector.memset(B_Q[:], 0.0)

            def bdw(kind, scale):
                return s_bd[:, _vidx(kind, scale), :]

            # ---------------- const DMAs
            nc.sync.dma_start(s_wsa[:], wsa[:])
            nc.sync.dma_start(s_bd[:], bdm[:])

            # ---------------- entry: fp16 transposed input DMAs + GEMMs
            # chain ch owns within-half columns [ch*NC2, (ch+1)*NC2)
            for ch in range(2):
                c0 = ch * NC2
                for k in range(NCH):
                    for src, dst in ((vtr, s_vt), (xtr, s_xt)):
                        for hb in range(2):
                            t0 = hb * NH + c0
                            nc.sync.dma_start(
                                dst[:, k, t0 : t0 + NC2],
                                src[k * 128 : (k + 1) * 128, t0 : t0 + NC2],
                            )
            for ch in range(2):
                c0 = ch * NC2
                for k in range(NCH):
                    last = k == NCH - 1
                    # (tsel, moving src, dest bank, dest cols)
                    for tsel, smov, bank, cols in (
                        (2, s_xt, B_h[ch], asl),  # h  (first: gates t1)
                        (1, s_vt, B_b[ch], asl),  # b  (gates m1)
                        (0, s_vt, B_a[ch], asl),  # a
                        (3, s_vt, B_a[ch], wsl),  # w0
                    ):
                        w_ap = s_wsa[:, tsel * NCH + k, :]
                        for hb in range(2):
                            t0 = hb * NH + c0
                            nc.tensor.matmul(
                                bank[hb * 64 : (hb + 1) * 64, cols],
                                w_ap,
                                smov[:, k, t0 : t0 + NC2],
                                start=False,
                                stop=last,
                                tile_position=(0, 64 * hb) if hb else None,
                                skip_group_check=True,
                            )

            # w0 -> fp16 (serves as nw for step 0)
            s_w0 = []
            for ch in range(2):
                w0t = consts.tile([128, NC2], F16, tag=f"w0_{ch}")
                nc.scalar.activation(w0t[:], B_a[ch][:, wsl], COPY)
                s_w0.append(w0t)

            # ---------------- token-major fp32 inputs (needed only at exit)
            for tb in range(NCH):
                nc.sync.dma_start(
                    s_vtok[:, tb, :], vin[tb * 128 : (tb + 1) * 128, :]
                )
                nc.sync.dma_start(
                    s_xtok[:, tb, :], xin[tb * 128 : (tb + 1) * 128, :]
                )
            nc.sync.dma_start(s_wcv[:], wcv[:])
            nc.sync.dma_start(s_wcx[:], wcx[:])
            # x0 += v0 in place on GPSIMD (idle during the steps)
            for tb in range(NCH):
                nc.gpsimd.tensor_add(
                    s_xtok[:, tb, :], s_xtok[:, tb, :], s_vtok[:, tb, :]
                )

            # ---------------- the 10 RK4 steps
            def mm(bank, sl, kind, scale, rhs, stop=False):
                nc.tensor.matmul(
                    bank[:, sl],
                    bdw(kind, scale),
                    rhs,
                    start=False,
                    stop=stop,
                    skip_group_check=True,
                )

            def step_chain(n, st):
                ch = st["ch"]
                sl = st["sl"]  # chain's columns in B_S/B_Q
                pa, pb, ph = B_a[ch], B_b[ch], B_h[ch]
                last = n == STEPS - 1
                q1, q23, q4 = float(10 - n), float(19 - 2 * n), float(9 - n)
                nw = st["nw"]

                def tanh():
                    t = tpool.tile([128, NC2], F16, tag=f"t{ch}")
                    nc.scalar.activation(t[:], ph[:, asl], TANH)
                    return t

                def prod(b_src, t_s):
                    m = mpool.tile([128, NC2], F16, tag=f"m{ch}")
                    nc.vector.tensor_mul(m[:], b_src, t_s[:])
                    c = cpool.tile([128, NC2], F16, tag=f"c{ch}")
                    nc.vector.tensor_mul(c[:], pa[:, asl], m[:])
                    return c

                # stage 1
                t1 = tanh()
                mm(ph, asl, "ibd", 1.0, nw[:], stop=True)  # h2
                t2 = tanh()
                c1 = prod(pb[:, asl], t1)
                mm(pb, asl, "cab", -DT / 2, c1[:], stop=True)  # b2
                mm(pa, asl, "caa", -DT / 2, c1[:], stop=True)  # a2
                mm(ph, asl, "cax", -D2 / 4, c1[:], stop=True)  # h3
                mm(B_S, sl, "ibd", 1.0, c1[:])
                mm(B_Q, sl, "ibd", q1, c1[:])
                yield

                # stage 2
                t3 = tanh()
                c2 = prod(pb[:, asl], t2)
                mm(pb, asl, "cab", DT / 2, c1[:])
                mm(pb, asl, "cab", -DT / 2, c2[:], stop=True)  # b3
                mm(pa, asl, "caa", DT / 2, c1[:])
                mm(pa, asl, "caa", -DT / 2, c2[:], stop=True)  # a3
                mm(ph, asl, "ibd", 1.0, nw[:])
                mm(ph, asl, "cax", D2 / 4, c1[:])
                mm(ph, asl, "cax", -D2 / 2, c2[:], stop=True)  # h4
                mm(B_S, sl, "ibd", 2.0, c2[:])
                mm(B_Q, sl, "ibd", q23, c2[:])
                yield

                # stage 3
                t4 = tanh()
                c3 = prod(pb[:, asl], t3)
                mm(pb, asl, "cab", DT / 2, c2[:])
                mm(pb, asl, "cab", -DT, c3[:], stop=True)  # b4
                mm(pa, asl, "caa", DT / 2, c2[:])
                mm(pa, asl, "caa", -DT, c3[:], stop=True)  # a4
                mm(B_S, sl, "ibd", 2.0, c3[:])
                mm(B_Q, sl, "ibd", q23, c3[:], stop=last)
                yield

                # stage 4
                c4 = prod(pb[:, asl], t4)
                mm(B_S, sl, "ibd", 1.0, c4[:], stop=last)
                if not last:
                    mm(B_Q, sl, "ibd", q4, c4[:])
                    mm(pb, asl, "cab", -DT / 6, c1[:])
                    mm(pb, asl, "cab", -DT / 3, c2[:])
                    mm(pb, asl, "cab", 2 * DT / 3, c3[:])
                    mm(pb, asl, "cab", -DT / 6, c4[:], stop=True)  # b1'
                    mm(ph, asl, "cax", -D2 / 6, c1[:])
                    mm(ph, asl, "cax", D2 / 3, c2[:])
                    mm(ph, asl, "cax", -D2 / 6, c3[:], stop=True)  # h1'
                    mm(pa, asl, "caa", -DT / 6, c1[:])
                    mm(pa, asl, "caa", -DT / 3, c2[:])
                    mm(pa, asl, "caa", 2 * DT / 3, c3[:])
                    mm(pa, asl, "caa", -DT / 6, c4[:], stop=True)  # a1'
                    # w update: w_{n+1} = w0 - (dt^2/12) Scum_{n+1} @ Cax
                    sc = spool.tile([128, NC2], F16, tag=f"sc{ch}")
                    nc.scalar.activation(sc[:], B_S[:, sl], COPY)
                    mm(pa, wsl, "cax", -D2 / 12, sc[:])
                    if st["sc_prev"] is not None:
                        mm(pa, wsl, "cax", D2 / 12, st["sc_prev"][:], stop=True)
                    st["sc_prev"] = sc
                    nwt = spool.tile([128, NC2], F16, tag=f"nw{ch}")
                    nc.vector.tensor_copy(nwt[:], pa[:, wsl])
                    st["nw"] = nwt
                yield

            def exit_chain(st):
                ch = st["ch"]
                sl = st["sl"]
                scf = epool.tile([128, NC2], F16, tag=f"scf{ch}")
                nc.scalar.activation(scf[:], B_S[:, sl], COPY)
                qcf = epool.tile([128, NC2], F16, tag=f"qcf{ch}")
                nc.scalar.activation(qcf[:], B_Q[:, sl], COPY)
                banks = [B_a[ch], B_b[ch], B_h[ch]]
                i = 0
                for th in range(2):
                    for tbl in range(2):
                        tb = th * 4 + 2 * ch + tbl
                        for dh in range(2):
                            dsl = slice(dh * NH, (dh + 1) * NH)
                            lhs_S = scf[th * 64 : (th + 1) * 64, tbl * 128 : (tbl + 1) * 128]
                            lhs_Q = qcf[th * 64 : (th + 1) * 64, tbl * 128 : (tbl + 1) * 128]
                            pv = banks[i % 3]
                            px = banks[(i + 1) % 3]
                            i += 2
                            nc.tensor.matmul(
                                pv[:],
                                lhs_S,
                                s_wcv[th * 64 : (th + 1) * 64, dsl],
                                start=True,
                                stop=True,
                                tile_position=(64 * th, 0),
                                skip_group_check=True,
                            )
                            ov = opool.tile([128, NH], F32, tag=f"ov{ch}")
                            nc.vector.tensor_add(ov[:], pv[:], s_vtok[:, tb, dsl])
                            nc.sync.dma_start(
                                vout[tb * 128 : (tb + 1) * 128, dsl], ov[:]
                            )
                            nc.tensor.matmul(
                                px[:],
                                lhs_Q,
                                s_wcx[th * 64 : (th + 1) * 64, dsl],
                                start=True,
                                stop=True,
                                tile_position=(64 * th, 0),
                                skip_group_check=True,
                            )
                            ox = opool.tile([128, NH], F32, tag=f"ox{ch}")
                            nc.vector.tensor_add(ox[:], px[:], s_xtok[:, tb, dsl])
                            nc.sync.dma_start(
                                xout[tb * 128 : (tb + 1) * 128, dsl], ox[:]
                            )
                        yield

            chains = [
                {"ch": c, "sl": slice(c * NC2, (c + 1) * NC2), "nw": s_w0[c], "sc_prev": None}
                for c in range(2)
            ]

            def chain_gen(st):
                for n in range(STEPS):
                    yield from step_chain(n, st)
                yield from exit_chain(st)

            gens = [chain_gen(st) for st in chains]
            alive = True
            while alive:
                alive = False
                for g in gens:
                    try:
                        next(g)
                        alive = True
                    except StopIteration:
                        pass

    orig = nc.to_json_bytes
    nc.to_json_bytes = lambda: _split_waits(orig())
    _NC_CACHE = nc
    return nc


# -------------------------------------------------------------------- driver


def _run(x, v, Wa, Wb, Wx, Wc, trace=False):
    from concourse.bass_utils import run_bass_kernel_spmd

    x = np.asarray(x, np.float32).reshape(BATCH * SEQ, DIM)
    v = np.asarray(v, np.float32).reshape(BATCH * SEQ, DIM)
    consts = _host_consts(Wa, Wb, Wx, Wc)

    nc = _build_bass()
    in_maps = []
    for c in range(NCORES):
        xc = np.ascontiguousarray(x[c * TPC : (c + 1) * TPC])
        vc = np.ascontiguousarray(v[c * TPC : (c + 1) * TPC])
        m = {
            "xin": xc,
            "vin": vc,
            "xt": np.ascontiguousarray(xc.T).astype(F16NP),
            "vt": np.ascontiguousarray(vc.T).astype(F16NP),
        }
        m.update(consts)
        in_maps.append(m)

    res = run_bass_kernel_spmd(
        nc, in_maps, core_ids=list(range(NCORES)), trace=trace
    )
    xo = np.concatenate([res.results[c]["xout"] for c in range(NCORES)], axis=0)
    vo = np.concatenate([res.results[c]["vout"] for c in range(NCORES)], axis=0)
    return (xo.reshape(BATCH, SEQ, DIM), vo.reshape(BATCH, SEQ, DIM)), res


def kernel(x, v, Wa, Wb, Wx, Wc):
    (xo, vo), _ = _run(x, v, Wa, Wb, Wx, Wc, trace=False)
    return xo, vo


# revision 5
# speedup vs baseline: 1.1884x; 1.0238x over previous
"""Trainium2 Bass kernel for nn_AdjointManifoldBlock.

Reference computes 10 RK4 steps of:
    dx/dt = v ; dv/dt = -gamma,  gamma = ((v@Wa)*(v@Wb)*tanh(x@Wx)) @ Wc

Rank-space restructuring (per token, rank=64 state):
    a = v@Wa, b = v@Wb, h = x@Wx, w = (dt/2) v@Wx
    c_s = a_s * b_s * tanh(h_s)   per RK4 stage
    every stage update is a [64,64] GEMM with Caa=Wc@Wa, Cab=Wc@Wb, Cax=Wc@Wx
    x_T = x0 + v0 - (dt^2/6) Q @ Wc,  v_T = v0 - (dt/6) S @ Wc
    S = sum S_n, Q = sum [(9-n) S_n + P_n]

Differences vs the earlier version of this kernel:
  - inputs are shipped twice: token-major fp32 (exit adds) and host-transposed
    feature-major fp16 (entry GEMMs) -> no PE transposes, no ACT copies
  - all step-loop matmul operands are fp16 (1 cycle/row at any width)
  - the b-chain lives in PSUM like a (PE undo-accumulation), killing the
    per-stage DVE badd; both per-stage products run on DVE (GPSIMD has no
    PSUM port)
  - the w-chain is derived from the S accumulator: w_n = w0 - (dt^2/12)
    Scum_n @ Cax, tracked in PSUM via per-step delta matmuls from fp16
    snapshots of S (2 matmuls/step instead of 4)
  - exit: x0+v0 pre-added on GPSIMD (idle otherwise) during the steps;
    final adds on DVE straight from PSUM; fp32 x,v DMA'd during the steps

Layout per core (1024 tokens): partition dim = [halfA ranks 0:64 | halfB
ranks 64:128], halves = tokens 0:512 / 512:1024; NSPLIT=2 column chains
(256 cols each) interleaved stage-by-stage for cross-engine overlap.
"""

import json
import numpy as np
import ml_dtypes

DIM = 1024
RANK = 64
STEPS = 10
DT = 0.1
BATCH, SEQ = 4, 2048
NCORES = 8
TPC = (BATCH * SEQ) // NCORES  # tokens per core = 1024
NH = TPC // 2  # tokens per stacked half = 512
NCH = DIM // 128  # feature chunks = 8
NSPLIT = 2
NC2 = NH // NSPLIT  # columns per chain = 256

D2 = DT * DT

CAA_SC = [-DT / 2, DT / 2, -DT, -DT / 6, -DT / 3, 2 * DT / 3]
CAB_SC = list(CAA_SC)
CAX_SC = [-D2 / 4, D2 / 4, -D2 / 2, D2 / 3, -D2 / 6, -D2 / 12, D2 / 12]
IBD_SC = sorted(
    {1.0, 2.0}
    | {float(10 - n) for n in range(STEPS)}
    | {float(19 - 2 * n) for n in range(STEPS)}
    | {float(9 - n) for n in range(STEPS) if 9 - n > 0}
)
NV = len(CAA_SC) + len(CAB_SC) + len(CAX_SC) + len(IBD_SC)

F16NP = np.float16


def _vidx(kind, scale):
    if kind == "caa":
        return CAA_SC.index(scale)
    if kind == "cab":
        return len(CAA_SC) + CAB_SC.index(scale)
    if kind == "cax":
        return len(CAA_SC) + len(CAB_SC) + CAX_SC.index(scale)
    if kind == "ibd":
        return len(CAA_SC) + len(CAB_SC) + len(CAX_SC) + IBD_SC.index(float(scale))
    raise KeyError(kind)


# ---------------------------------------------------------------- host consts


def _host_consts(Wa, Wb, Wx, Wc):
    Wa64 = np.asarray(Wa, np.float64)
    Wb64 = np.asarray(Wb, np.float64)
    Wx64 = np.asarray(Wx, np.float64)
    Wc64 = np.asarray(Wc, np.float64)

    Caa = Wc64 @ Wa64  # [64, 64]; row index = contraction side
    Cab = Wc64 @ Wb64
    Cax = Wc64 @ Wx64
    I64 = np.eye(RANK)

    mats = (
        [Caa * s for s in CAA_SC]
        + [Cab * s for s in CAB_SC]
        + [Cax * s for s in CAX_SC]
        + [I64 * s for s in IBD_SC]
    )
    bd = np.zeros((NV, 128, 128), np.float64)
    for i, m in enumerate(mats):
        bd[i, 0:64, 0:64] = m
        bd[i, 64:128, 64:128] = m
    bd = np.ascontiguousarray(bd.transpose(1, 0, 2)).astype(F16NP)  # [128, NV, 128]

    stk = np.stack(
        [W.reshape(NCH, 128, RANK) for W in (Wa64, Wb64, Wx64, (DT / 2) * Wx64)]
    )  # [4, 8, 128, 64]
    wsa = np.ascontiguousarray(stk.transpose(2, 0, 1, 3).reshape(128, 4 * NCH, RANK)).astype(
        F16NP
    )
    wcv1 = -(DT / 6) * Wc64  # [64, 1024]
    wcx1 = -(D2 / 6) * Wc64
    wcv = np.concatenate([wcv1, wcv1], axis=0).astype(F16NP)  # [128, 1024] duplicated
    wcx = np.concatenate([wcx1, wcx1], axis=0).astype(F16NP)

    return {"bd": bd, "wsa": wsa, "wcv": wcv, "wcx": wcx}


# ----------------------------------------------------------- BIR wait postpass


def _split_waits(data: bytes) -> bytes:
    """This walrus build accepts only one inline sync wait per instruction;
    move excess waits onto NoOps inserted before the instruction (the
    engine sequencer processes them in order, so semantics are identical)."""
    bir = json.loads(data)
    for fn in bir["functions"]:
        for blk in fn["blocks"]:
            out = []
            k = 0
            for inst in blk["instructions"]:
                si = inst.get("sync_info")
                if si and len(si.get("on_wait", [])) > 1:
                    waits = si["on_wait"]
                    pre = []
                    while len(waits) > 1:
                        chunk, waits = waits[:1], waits[1:]
                        k += 1
                        pre.append(
                            {
                                "name": f"{inst['name']}-w{k}",
                                "opcode": "NoOp",
                                "engine": inst["engine"],
                                "ins": [],
                                "outs": [],
                                "sync_info": {"on_wait": chunk, "on_update": []},
                            }
                        )
                    si["on_wait"] = waits
                    out.extend(pre)
                out.append(inst)
            blk["instructions"] = out
    return json.dumps(bir).encode()


# ---------------------------------------------------------------- bass builder

_NC_CACHE = None


def _build_bass():
    global _NC_CACHE
    if _NC_CACHE is not None:
        return _NC_CACHE

    import concourse.bass as bass
    import concourse.tile as tile
    import concourse.mybir as mybir

    F32 = mybir.dt.float32
    F16 = mybir.dt.float16
    TANH = mybir.ActivationFunctionType.Tanh
    COPY = mybir.ActivationFunctionType.Copy

    nc = bass.Bass("TRN2", target_bir_lowering=False, debug=False, num_devices=1)

    xin = nc.dram_tensor("xin", [TPC, DIM], F16, kind="ExternalInput").ap()
    vin = nc.dram_tensor("vin", [TPC, DIM], F16, kind="ExternalInput").ap()
    xtr = nc.dram_tensor("xt", [DIM, TPC], F16, kind="ExternalInput").ap()
    vtr = nc.dram_tensor("vt", [DIM, TPC], F16, kind="ExternalInput").ap()
    bdm = nc.dram_tensor("bd", [128, NV, 128], F16, kind="ExternalInput").ap()
    wsa = nc.dram_tensor("wsa", [128, 4 * NCH, RANK], F16, kind="ExternalInput").ap()
    wcv = nc.dram_tensor("wcv", [128, DIM], F16, kind="ExternalInput").ap()
    wcx = nc.dram_tensor("wcx", [128, DIM], F16, kind="ExternalInput").ap()
    xout = nc.dram_tensor("xout", [TPC, DIM], F16, kind="ExternalOutput").ap()
    vout = nc.dram_tensor("vout", [TPC, DIM], F16, kind="ExternalOutput").ap()

    with tile.TileContext(nc) as tc:
        with (
            tc.tile_pool(name="consts", bufs=1) as consts,
            tc.tile_pool(name="tpool", bufs=6) as tpool,
            tc.tile_pool(name="mpool", bufs=4) as mpool,
            tc.tile_pool(name="cpool", bufs=6) as cpool,
            tc.tile_pool(name="spool", bufs=3) as spool,
            tc.tile_pool(name="epool", bufs=1) as epool,
            tc.tile_pool(name="opool", bufs=4) as opool,
            tc.tile_pool(name="ps", bufs=1, space="PSUM") as ps,
        ):
            # ---------------- tiles
            s_bd = consts.tile([128, NV, 128], F16, tag="bd")
            s_wsa = consts.tile([128, 4 * NCH, RANK], F16, tag="wsa")
            s_wcv = consts.tile([128, DIM], F16, tag="wcv")
            s_wcx = consts.tile([128, DIM], F16, tag="wcx")
            s_vt = consts.tile([128, NCH, TPC], F16, tag="vt")
            s_xt = consts.tile([128, NCH, TPC], F16, tag="xt")
            s_vtok = consts.tile([128, NCH, DIM], F16, tag="vtok")
            s_xtok = consts.tile([128, NCH, DIM], F16, tag="xtok")

            B_a = [ps.tile([128, 2 * NC2], F32, tag=f"Ba{c}", name=f"Ba{c}") for c in range(2)]
            B_b = [ps.tile([128, 2 * NC2], F32, tag=f"Bb{c}", name=f"Bb{c}") for c in range(2)]
            B_h = [ps.tile([128, 2 * NC2], F32, tag=f"Bh{c}", name=f"Bh{c}") for c in range(2)]
            B_S = ps.tile([128, NH], F32, tag="BS")
            B_Q = ps.tile([128, NH], F32, tag="BQ")

            asl = slice(0, NC2)  # a/b/h state columns within chain banks
            wsl = slice(NC2, 2 * NC2)  # w columns within B_a

            # memsets: a-banks fully (w region too); b/h state region only
            for c in range(2):
                nc.vector.memset(B_a[c][:], 0.0)
                nc.vector.memset(B_b[c][:, asl], 0.0)
                nc.vector.memset(B_h[c][:, asl], 0.0)
            nc.vector.memset(B_S[:], 0.0)
            nc.vector.memset(B_Q[:], 0.0)

            def bdw(kind, scale):
                return s_bd[:, _vidx(kind, scale), :]

            # ---------------- const DMAs
            nc.sync.dma_start(s_wsa[:], wsa[:])
            nc.sync.dma_start(s_bd[:], bdm[:])

            # ---------------- entry: fp16 transposed input DMAs + GEMMs
            # chain ch owns within-half columns [ch*NC2, (ch+1)*NC2)
            for ch in range(2):
                c0 = ch * NC2
                for k in range(NCH):
                    for src, dst in ((vtr, s_vt), (xtr, s_xt)):
                        for hb in range(2):
                            t0 = hb * NH + c0
                            nc.sync.dma_start(
                                dst[:, k, t0 : t0 + NC2],
                                src[k * 128 : (k + 1) * 128, t0 : t0 + NC2],
                            )
            for ch in range(2):
                c0 = ch * NC2
                for k in range(NCH):
                    last = k == NCH - 1
                    # (tsel, moving src, dest bank, dest cols)
                    for tsel, smov, bank, cols in (
                        (2, s_xt, B_h[ch], asl),  # h  (first: gates t1)
                        (1, s_vt, B_b[ch], asl),  # b  (gates m1)
                        (0, s_vt, B_a[ch], asl),  # a
                        (3, s_vt, B_a[ch], wsl),  # w0
                    ):
                        w_ap = s_wsa[:, tsel * NCH + k, :]
                        for hb in range(2):
                            t0 = hb * NH + c0
                            nc.tensor.matmul(
                                bank[hb * 64 : (hb + 1) * 64, cols],
                                w_ap,
                                smov[:, k, t0 : t0 + NC2],
                                start=False,
                                stop=last,
                                tile_position=(0, 64 * hb) if hb else None,
                                skip_group_check=True,
                            )

            # w0 -> fp16 (serves as nw for step 0)
            s_w0 = []
            for ch in range(2):
                w0t = consts.tile([128, NC2], F16, tag=f"w0_{ch}")
                nc.scalar.activation(w0t[:], B_a[ch][:, wsl], COPY)
                s_w0.append(w0t)

            # ---------------- token-major fp32 inputs (needed only at exit)
            for tb in range(NCH):
                nc.sync.dma_start(
                    s_vtok[:, tb, :], vin[tb * 128 : (tb + 1) * 128, :]
                )
                nc.sync.dma_start(
                    s_xtok[:, tb, :], xin[tb * 128 : (tb + 1) * 128, :]
                )
            nc.sync.dma_start(s_wcv[:], wcv[:])
            nc.sync.dma_start(s_wcx[:], wcx[:])
            # x0 += v0 in place on GPSIMD (idle during the steps)
            for tb in range(NCH):
                nc.gpsimd.tensor_add(
                    s_xtok[:, tb, :], s_xtok[:, tb, :], s_vtok[:, tb, :]
                )

            # ---------------- the 10 RK4 steps
            def mm(bank, sl, kind, scale, rhs, stop=False):
                nc.tensor.matmul(
                    bank[:, sl],
                    bdw(kind, scale),
                    rhs,
                    start=False,
                    stop=stop,
                    skip_group_check=True,
                )

            def step_chain(n, st):
                ch = st["ch"]
                sl = st["sl"]  # chain's columns in B_S/B_Q
                pa, pb, ph = B_a[ch], B_b[ch], B_h[ch]
                last = n == STEPS - 1
                q1, q23, q4 = float(10 - n), float(19 - 2 * n), float(9 - n)
                nw = st["nw"]

                def tanh():
                    t = tpool.tile([128, NC2], F16, tag=f"t{ch}")
                    nc.scalar.activation(t[:], ph[:, asl], TANH)
                    return t

                def prod(b_src, t_s):
                    m = mpool.tile([128, NC2], F16, tag=f"m{ch}")
                    nc.vector.tensor_mul(m[:], b_src, t_s[:])
                    c = cpool.tile([128, NC2], F16, tag=f"c{ch}")
                    nc.vector.tensor_mul(c[:], pa[:, asl], m[:])
                    return c

                # stage 1
                t1 = tanh()
                mm(ph, asl, "ibd", 1.0, nw[:], stop=True)  # h2
                t2 = tanh()
                c1 = prod(pb[:, asl], t1)
                mm(pb, asl, "cab", -DT / 2, c1[:], stop=True)  # b2
                mm(pa, asl, "caa", -DT / 2, c1[:], stop=True)  # a2
                mm(ph, asl, "cax", -D2 / 4, c1[:], stop=True)  # h3
                mm(B_S, sl, "ibd", 1.0, c1[:])
                mm(B_Q, sl, "ibd", q1, c1[:])
                yield

                # stage 2
                t3 = tanh()
                c2 = prod(pb[:, asl], t2)
                mm(pb, asl, "cab", DT / 2, c1[:])
                mm(pb, asl, "cab", -DT / 2, c2[:], stop=True)  # b3
                mm(pa, asl, "caa", DT / 2, c1[:])
                mm(pa, asl, "caa", -DT / 2, c2[:], stop=True)  # a3
                mm(ph, asl, "ibd", 1.0, nw[:])
                mm(ph, asl, "cax", D2 / 4, c1[:])
                mm(ph, asl, "cax", -D2 / 2, c2[:], stop=True)  # h4
                mm(B_S, sl, "ibd", 2.0, c2[:])
                mm(B_Q, sl, "ibd", q23, c2[:])
                yield

                # stage 3
                t4 = tanh()
                c3 = prod(pb[:, asl], t3)
                mm(pb, asl, "cab", DT / 2, c2[:])
                mm(pb, asl, "cab", -DT, c3[:], stop=True)  # b4
                mm(pa, asl, "caa", DT / 2, c2[:])
                mm(pa, asl, "caa", -DT, c3[:], stop=True)  # a4
                mm(B_S, sl, "ibd", 2.0, c3[:])
                mm(B_Q, sl, "ibd", q23, c3[:], stop=last)
                yield

                # stage 4
                c4 = prod(pb[:, asl], t4)
                mm(B_S, sl, "ibd", 1.0, c4[:], stop=last)
                if not last:
                    mm(B_Q, sl, "ibd", q4, c4[:])
                    mm(pb, asl, "cab", -DT / 6, c1[:])
                    mm(pb, asl, "cab", -DT / 3, c2[:])
                    mm(pb, asl, "cab", 2 * DT / 3, c3[:])
                    mm(pb, asl, "cab", -DT / 6, c4[:], stop=True)  # b1'
                    mm(ph, asl, "cax", -D2 / 6, c1[:])
                    mm(ph, asl, "cax", D2 / 3, c2[:])
                    mm(ph, asl, "cax", -D2 / 6, c3[:], stop=True)  # h1'
                    mm(pa, asl, "caa", -DT / 6, c1[:])
                    mm(pa, asl, "caa", -DT / 3, c2[:])
                    mm(pa, asl, "caa", 2 * DT / 3, c3[:])
                    mm(pa, asl, "caa", -DT / 6, c4[:], stop=True)  # a1'
                    # w update: w_{n+1} = w0 - (dt^2/12) Scum_{n+1} @ Cax
                    sc = spool.tile([128, NC2], F16, tag=f"sc{ch}")
                    nc.scalar.activation(sc[:], B_S[:, sl], COPY)
                    mm(pa, wsl, "cax", -D2 / 12, sc[:])
                    if st["sc_prev"] is not None:
                        mm(pa, wsl, "cax", D2 / 12, st["sc_prev"][:], stop=True)
                    st["sc_prev"] = sc
                    nwt = spool.tile([128, NC2], F16, tag=f"nw{ch}")
                    nc.scalar.activation(nwt[:], pa[:, wsl], COPY)
                    st["nw"] = nwt
                yield

            def exit_chain(st):
                ch = st["ch"]
                sl = st["sl"]
                scf = epool.tile([128, NC2], F16, tag=f"scf{ch}")
                nc.scalar.activation(scf[:], B_S[:, sl], COPY)
                qcf = epool.tile([128, NC2], F16, tag=f"qcf{ch}")
                nc.scalar.activation(qcf[:], B_Q[:, sl], COPY)
                banks = [B_a[ch], B_b[ch], B_h[ch]]
                i = 0
                for th in range(2):
                    for tbl in range(2):
                        tb = th * 4 + 2 * ch + tbl
                        for dh in range(2):
                            dsl = slice(dh * NH, (dh + 1) * NH)
                            lhs_S = scf[th * 64 : (th + 1) * 64, tbl * 128 : (tbl + 1) * 128]
                            lhs_Q = qcf[th * 64 : (th + 1) * 64, tbl * 128 : (tbl + 1) * 128]
                            pv = banks[i % 3]
                            px = banks[(i + 1) % 3]
                            i += 2
                            nc.tensor.matmul(
                                pv[:],
                                lhs_S,
                                s_wcv[th * 64 : (th + 1) * 64, dsl],
                                start=True,
                                stop=True,
                                tile_position=(64 * th, 0),
                                skip_group_check=True,
                            )
                            ov = opool.tile([128, NH], F16, tag=f"ov{ch}")
                            nc.vector.tensor_add(ov[:], pv[:], s_vtok[:, tb, dsl])
                            nc.sync.dma_start(
                                vout[tb * 128 : (tb + 1) * 128, dsl], ov[:]
                            )
                            nc.tensor.matmul(
                                px[:],
                                lhs_Q,
                                s_wcx[th * 64 : (th + 1) * 64, dsl],
                                start=True,
                                stop=True,
                                tile_position=(64 * th, 0),
                                skip_group_check=True,
                            )
                            ox = opool.tile([128, NH], F16, tag=f"ox{ch}")
                            nc.vector.tensor_add(ox[:], px[:], s_xtok[:, tb, dsl])
                            nc.sync.dma_start(
                                xout[tb * 128 : (tb + 1) * 128, dsl], ox[:]
                            )
                        yield

            chains = [
                {"ch": c, "sl": slice(c * NC2, (c + 1) * NC2), "nw": s_w0[c], "sc_prev": None}
                for c in range(2)
            ]

            def chain_gen(st):
                for n in range(STEPS):
                    yield from step_chain(n, st)
                yield from exit_chain(st)

            gens = [chain_gen(st) for st in chains]
            alive = True
            while alive:
                alive = False
                for g in gens:
                    try:
                        next(g)
                        alive = True
                    except StopIteration:
                        pass

    orig = nc.to_json_bytes
    nc.to_json_bytes = lambda: _split_waits(orig())
    _NC_CACHE = nc
    return nc


# -------------------------------------------------------------------- driver


def _run(x, v, Wa, Wb, Wx, Wc, trace=False):
    from concourse.bass_utils import run_bass_kernel_spmd

    x = np.asarray(x, np.float32).reshape(BATCH * SEQ, DIM)
    v = np.asarray(v, np.float32).reshape(BATCH * SEQ, DIM)
    consts = _host_consts(Wa, Wb, Wx, Wc)

    nc = _build_bass()
    in_maps = []
    for c in range(NCORES):
        xc = np.ascontiguousarray(x[c * TPC : (c + 1) * TPC])
        vc = np.ascontiguousarray(v[c * TPC : (c + 1) * TPC])
        m = {
            "xin": xc.astype(F16NP),
            "vin": vc.astype(F16NP),
            "xt": np.ascontiguousarray(xc.T).astype(F16NP),
            "vt": np.ascontiguousarray(vc.T).astype(F16NP),
        }
        m.update(consts)
        in_maps.append(m)

    res = run_bass_kernel_spmd(
        nc, in_maps, core_ids=list(range(NCORES)), trace=trace
    )
    xo = np.concatenate(
        [np.asarray(res.results[c]["xout"], np.float32) for c in range(NCORES)], axis=0
    )
    vo = np.concatenate(
        [np.asarray(res.results[c]["vout"], np.float32) for c in range(NCORES)], axis=0
    )
    return (xo.reshape(BATCH, SEQ, DIM), vo.reshape(BATCH, SEQ, DIM)), res


def kernel(x, v, Wa, Wb, Wx, Wc):
    (xo, vo), _ = _run(x, v, Wa, Wb, Wx, Wc, trace=False)
    return xo, vo


# revision 7
# speedup vs baseline: 1.3878x; 1.1678x over previous
"""Trainium2 Bass kernel for nn_AdjointManifoldBlock.

Reference computes 10 RK4 steps of:
    dx/dt = v ; dv/dt = -gamma,  gamma = ((v@Wa)*(v@Wb)*tanh(x@Wx)) @ Wc

Rank-space restructuring (per token, rank=64 state):
    a = v@Wa, b = v@Wb, h = x@Wx, w = (dt/2) v@Wx
    c_s = a_s * b_s * tanh(h_s)   per RK4 stage
    every stage update is a [64,64] GEMM with Caa=Wc@Wa, Cab=Wc@Wb, Cax=Wc@Wx
    x_T = x0 + v0 - (dt^2/6) Q @ Wc,  v_T = v0 - (dt/6) S @ Wc
    S = sum S_n, Q = sum [(9-n) S_n + P_n]

Differences vs the earlier version of this kernel:
  - inputs are shipped twice: token-major fp32 (exit adds) and host-transposed
    feature-major fp16 (entry GEMMs) -> no PE transposes, no ACT copies
  - all step-loop matmul operands are fp16 (1 cycle/row at any width)
  - the b-chain lives in PSUM like a (PE undo-accumulation), killing the
    per-stage DVE badd; both per-stage products run on DVE (GPSIMD has no
    PSUM port)
  - the w-chain is derived from the S accumulator: w_n = w0 - (dt^2/12)
    Scum_n @ Cax, tracked in PSUM via per-step delta matmuls from fp16
    snapshots of S (2 matmuls/step instead of 4)
  - exit: x0+v0 pre-added on GPSIMD (idle otherwise) during the steps;
    final adds on DVE straight from PSUM; fp32 x,v DMA'd during the steps

Layout per core (1024 tokens): partition dim = [halfA ranks 0:64 | halfB
ranks 64:128], halves = tokens 0:512 / 512:1024; NSPLIT=2 column chains
(256 cols each) interleaved stage-by-stage for cross-engine overlap.
"""

import json
import numpy as np
import ml_dtypes

DIM = 1024
RANK = 64
STEPS = 10
DT = 0.1
BATCH, SEQ = 4, 2048
NCORES = 8
TPC = (BATCH * SEQ) // NCORES  # tokens per core = 1024
NH = TPC // 2  # tokens per stacked half = 512
NCH = DIM // 128  # feature chunks = 8
NSPLIT = 2
NC2 = NH // NSPLIT  # columns per chain = 256

D2 = DT * DT

CAA_SC = [-DT / 2, DT / 2, -DT, -DT / 6, -DT / 3, 2 * DT / 3]
CAB_SC = list(CAA_SC)
CAX_SC = [-D2 / 4, D2 / 4, -D2 / 2, D2 / 3, -D2 / 6, -D2 / 12, D2 / 12]
IBD_SC = sorted(
    {1.0, 2.0}
    | {float(10 - n) for n in range(STEPS)}
    | {float(19 - 2 * n) for n in range(STEPS)}
    | {float(9 - n) for n in range(STEPS) if 9 - n > 0}
)
NV = len(CAA_SC) + len(CAB_SC) + len(CAX_SC) + len(IBD_SC)

F16NP = np.float16


def _vidx(kind, scale):
    if kind == "caa":
        return CAA_SC.index(scale)
    if kind == "cab":
        return len(CAA_SC) + CAB_SC.index(scale)
    if kind == "cax":
        return len(CAA_SC) + len(CAB_SC) + CAX_SC.index(scale)
    if kind == "ibd":
        return len(CAA_SC) + len(CAB_SC) + len(CAX_SC) + IBD_SC.index(float(scale))
    raise KeyError(kind)


# ---------------------------------------------------------------- host consts


def _host_consts(Wa, Wb, Wx, Wc):
    Wa64 = np.asarray(Wa, np.float64)
    Wb64 = np.asarray(Wb, np.float64)
    Wx64 = np.asarray(Wx, np.float64)
    Wc64 = np.asarray(Wc, np.float64)

    Caa = Wc64 @ Wa64  # [64, 64]; row index = contraction side
    Cab = Wc64 @ Wb64
    Cax = Wc64 @ Wx64
    I64 = np.eye(RANK)

    mats = (
        [Caa * s for s in CAA_SC]
        + [Cab * s for s in CAB_SC]
        + [Cax * s for s in CAX_SC]
        + [I64 * s for s in IBD_SC]
    )
    bd = np.zeros((NV, 128, 128), np.float64)
    for i, m in enumerate(mats):
        bd[i, 0:64, 0:64] = m
        bd[i, 64:128, 64:128] = m
    bd = np.ascontiguousarray(bd.transpose(1, 0, 2)).astype(F16NP)  # [128, NV, 128]

    stk = np.stack(
        [W.reshape(NCH, 128, RANK) for W in (Wa64, Wb64, Wx64, (DT / 2) * Wx64)]
    )  # [4, 8, 128, 64]
    wsa = np.ascontiguousarray(stk.transpose(2, 0, 1, 3).reshape(128, 4 * NCH, RANK)).astype(
        F16NP
    )
    wcv1 = -(DT / 6) * Wc64  # [64, 1024]
    wcx1 = -(D2 / 6) * Wc64
    wcv = np.concatenate([wcv1, wcv1], axis=0).astype(F16NP)  # [128, 1024] duplicated
    wcx = np.concatenate([wcx1, wcx1], axis=0).astype(F16NP)

    return {"bd": bd, "wsa": wsa, "wcv": wcv, "wcx": wcx}


# ----------------------------------------------------------- BIR wait postpass


def _split_waits(data: bytes) -> bytes:
    """This walrus build accepts only one inline sync wait per instruction;
    move excess waits onto NoOps inserted before the instruction (the
    engine sequencer processes them in order, so semantics are identical)."""
    bir = json.loads(data)
    for fn in bir["functions"]:
        for blk in fn["blocks"]:
            out = []
            k = 0
            for inst in blk["instructions"]:
                si = inst.get("sync_info")
                if si and len(si.get("on_wait", [])) > 1:
                    waits = si["on_wait"]
                    pre = []
                    while len(waits) > 1:
                        chunk, waits = waits[:1], waits[1:]
                        k += 1
                        pre.append(
                            {
                                "name": f"{inst['name']}-w{k}",
                                "opcode": "NoOp",
                                "engine": inst["engine"],
                                "ins": [],
                                "outs": [],
                                "sync_info": {"on_wait": chunk, "on_update": []},
                            }
                        )
                    si["on_wait"] = waits
                    out.extend(pre)
                out.append(inst)
            blk["instructions"] = out
    return json.dumps(bir).encode()


# ---------------------------------------------------------------- bass builder

_NC_CACHE = None


def _build_bass():
    global _NC_CACHE
    if _NC_CACHE is not None:
        return _NC_CACHE

    import concourse.bass as bass
    import concourse.tile as tile
    import concourse.mybir as mybir

    F32 = mybir.dt.float32
    F16 = mybir.dt.float16
    TANH = mybir.ActivationFunctionType.Tanh
    COPY = mybir.ActivationFunctionType.Copy

    nc = bass.Bass("TRN2", target_bir_lowering=False, debug=False, num_devices=1)

    xin = nc.dram_tensor("xin", [TPC, DIM], F16, kind="ExternalInput").ap()
    vin = nc.dram_tensor("vin", [TPC, DIM], F16, kind="ExternalInput").ap()
    xtr = nc.dram_tensor("xt", [DIM, TPC], F16, kind="ExternalInput").ap()
    vtr = nc.dram_tensor("vt", [DIM, TPC], F16, kind="ExternalInput").ap()
    bdm = nc.dram_tensor("bd", [128, NV, 128], F16, kind="ExternalInput").ap()
    wsa = nc.dram_tensor("wsa", [128, 4 * NCH, RANK], F16, kind="ExternalInput").ap()
    wcv = nc.dram_tensor("wcv", [128, DIM], F16, kind="ExternalInput").ap()
    wcx = nc.dram_tensor("wcx", [128, DIM], F16, kind="ExternalInput").ap()
    xout = nc.dram_tensor("xout", [TPC, DIM], F16, kind="ExternalOutput").ap()
    vout = nc.dram_tensor("vout", [TPC, DIM], F16, kind="ExternalOutput").ap()

    with tile.TileContext(nc) as tc:
        with (
            tc.tile_pool(name="consts", bufs=1) as consts,
            tc.tile_pool(name="tpool", bufs=6) as tpool,
            tc.tile_pool(name="mpool", bufs=4) as mpool,
            tc.tile_pool(name="cpool", bufs=6) as cpool,
            tc.tile_pool(name="spool", bufs=3) as spool,
            tc.tile_pool(name="epool", bufs=1) as epool,
            tc.tile_pool(name="opool", bufs=4) as opool,
            tc.tile_pool(name="ps", bufs=1, space="PSUM") as ps,
        ):
            # ---------------- tiles
            s_bd = consts.tile([128, NV, 128], F16, tag="bd")
            s_wsa = consts.tile([128, 4 * NCH, RANK], F16, tag="wsa")
            s_wcv = consts.tile([128, DIM], F16, tag="wcv")
            s_wcx = consts.tile([128, DIM], F16, tag="wcx")
            s_vt = consts.tile([128, NCH, TPC], F16, tag="vt")
            s_xt = consts.tile([128, NCH, TPC], F16, tag="xt")
            s_vtok = consts.tile([128, NCH, DIM], F16, tag="vtok")
            s_xtok = consts.tile([128, NCH, DIM], F16, tag="xtok")

            B_a = [ps.tile([128, 2 * NC2], F32, tag=f"Ba{c}", name=f"Ba{c}") for c in range(2)]
            B_b = [ps.tile([128, 2 * NC2], F32, tag=f"Bb{c}", name=f"Bb{c}") for c in range(2)]
            B_h = [ps.tile([128, 2 * NC2], F32, tag=f"Bh{c}", name=f"Bh{c}") for c in range(2)]
            B_S = ps.tile([128, NH], F32, tag="BS")
            B_Q = ps.tile([128, NH], F32, tag="BQ")

            asl = slice(0, NC2)  # a/b/h state columns within chain banks
            wsl = slice(NC2, 2 * NC2)  # w columns within B_a

            # memsets: a-banks fully (w region too); b/h state region only
            for c in range(2):
                nc.vector.memset(B_a[c][:], 0.0)
                nc.vector.memset(B_b[c][:, asl], 0.0)
                nc.vector.memset(B_h[c][:, asl], 0.0)
            nc.vector.memset(B_S[:], 0.0)
            nc.vector.memset(B_Q[:], 0.0)

            def bdw(kind, scale):
                return s_bd[:, _vidx(kind, scale), :]

            # ---------------- const DMAs
            nc.sync.dma_start(s_wsa[:], wsa[:])
            nc.sync.dma_start(s_bd[:], bdm[:])

            # ---------------- entry: fp16 transposed input DMAs + GEMMs
            # chain ch owns within-half columns [ch*NC2, (ch+1)*NC2)
            # one strided DMA per (tensor, chain): all 8 chunks, both halves
            for ch in range(2):
                c0 = ch * NC2
                for src, dst in ((vtr, s_vt), (xtr, s_xt)):
                    sv = src.rearrange("(k p) c -> p k c", p=128)
                    for hb in range(2):
                        t0 = hb * NH + c0
                        nc.sync.dma_start(
                            dst[:, :, t0 : t0 + NC2], sv[:, :, t0 : t0 + NC2]
                        )
            for ch in range(2):
                c0 = ch * NC2
                for k in range(NCH):
                    last = k == NCH - 1
                    # (tsel, moving src, dest bank, dest cols)
                    for tsel, smov, bank, cols in (
                        (2, s_xt, B_h[ch], asl),  # h  (first: gates t1)
                        (1, s_vt, B_b[ch], asl),  # b  (gates m1)
                        (0, s_vt, B_a[ch], asl),  # a
                        (3, s_vt, B_a[ch], wsl),  # w0
                    ):
                        w_ap = s_wsa[:, tsel * NCH + k, :]
                        for hb in range(2):
                            t0 = hb * NH + c0
                            nc.tensor.matmul(
                                bank[hb * 64 : (hb + 1) * 64, cols],
                                w_ap,
                                smov[:, k, t0 : t0 + NC2],
                                start=False,
                                stop=last,
                                tile_position=(0, 64 * hb) if hb else None,
                                skip_group_check=True,
                            )

            # w0 -> fp16 (serves as nw for step 0)
            s_w0 = []
            for ch in range(2):
                w0t = consts.tile([128, NC2], F16, tag=f"w0_{ch}")
                nc.scalar.activation(w0t[:], B_a[ch][:, wsl], COPY)
                s_w0.append(w0t)

            # ---------------- token-major fp32 inputs (needed only at exit)
            nc.sync.dma_start(
                s_vtok[:], vin.rearrange("(tb p) c -> p tb c", p=128)
            )
            nc.sync.dma_start(
                s_xtok[:], xin.rearrange("(tb p) c -> p tb c", p=128)
            )
            nc.sync.dma_start(s_wcv[:], wcv[:])
            nc.sync.dma_start(s_wcx[:], wcx[:])
            # x0 += v0 in place on GPSIMD (idle during the steps)
            for tb in range(NCH):
                nc.gpsimd.tensor_add(
                    s_xtok[:, tb, :], s_xtok[:, tb, :], s_vtok[:, tb, :]
                )

            # ---------------- the 10 RK4 steps
            def mm(bank, sl, kind, scale, rhs, stop=False):
                nc.tensor.matmul(
                    bank[:, sl],
                    bdw(kind, scale),
                    rhs,
                    start=False,
                    stop=stop,
                    skip_group_check=True,
                )

            def step_chain(n, st):
                ch = st["ch"]
                sl = st["sl"]  # chain's columns in B_S/B_Q
                pa, pb, ph = B_a[ch], B_b[ch], B_h[ch]
                last = n == STEPS - 1
                q1, q23, q4 = float(10 - n), float(19 - 2 * n), float(9 - n)
                nw = st["nw"]

                def tanh():
                    t = tpool.tile([128, NC2], F16, tag=f"t{ch}")
                    nc.scalar.activation(t[:], ph[:, asl], TANH)
                    return t

                def prod(b_src, t_s):
                    m = mpool.tile([128, NC2], F16, tag=f"m{ch}")
                    nc.vector.tensor_mul(m[:], b_src, t_s[:])
                    c = cpool.tile([128, NC2], F16, tag=f"c{ch}")
                    nc.vector.tensor_mul(c[:], pa[:, asl], m[:])
                    return c

                # stage 1
                t1 = tanh()
                mm(ph, asl, "ibd", 1.0, nw[:], stop=True)  # h2
                t2 = tanh()
                c1 = prod(pb[:, asl], t1)
                mm(pb, asl, "cab", -DT / 2, c1[:], stop=True)  # b2
                mm(pa, asl, "caa", -DT / 2, c1[:], stop=True)  # a2
                mm(ph, asl, "cax", -D2 / 4, c1[:], stop=True)  # h3
                mm(B_S, sl, "ibd", 1.0, c1[:])
                mm(B_Q, sl, "ibd", q1, c1[:])
                yield

                # stage 2
                t3 = tanh()
                c2 = prod(pb[:, asl], t2)
                mm(pb, asl, "cab", DT / 2, c1[:])
                mm(pb, asl, "cab", -DT / 2, c2[:], stop=True)  # b3
                mm(pa, asl, "caa", DT / 2, c1[:])
                mm(pa, asl, "caa", -DT / 2, c2[:], stop=True)  # a3
                mm(ph, asl, "ibd", 1.0, nw[:])
                mm(ph, asl, "cax", D2 / 4, c1[:])
                mm(ph, asl, "cax", -D2 / 2, c2[:], stop=True)  # h4
                mm(B_S, sl, "ibd", 2.0, c2[:])
                mm(B_Q, sl, "ibd", q23, c2[:])
                yield

                # stage 3
                t4 = tanh()
                c3 = prod(pb[:, asl], t3)
                mm(pb, asl, "cab", DT / 2, c2[:])
                mm(pb, asl, "cab", -DT, c3[:], stop=True)  # b4
                mm(pa, asl, "caa", DT / 2, c2[:])
                mm(pa, asl, "caa", -DT, c3[:], stop=True)  # a4
                mm(B_S, sl, "ibd", 2.0, c3[:])
                mm(B_Q, sl, "ibd", q23, c3[:], stop=last)
                yield

                # stage 4
                c4 = prod(pb[:, asl], t4)
                if not last:
                    mm(pb, asl, "cab", -DT / 6, c1[:])
                    mm(pb, asl, "cab", -DT / 3, c2[:])
                    mm(pb, asl, "cab", 2 * DT / 3, c3[:])
                    mm(pb, asl, "cab", -DT / 6, c4[:], stop=True)  # b1'
                    mm(ph, asl, "cax", -D2 / 6, c1[:])
                    mm(ph, asl, "cax", D2 / 3, c2[:])
                    mm(ph, asl, "cax", -D2 / 6, c3[:], stop=True)  # h1'
                    mm(pa, asl, "caa", -DT / 6, c1[:])
                    mm(pa, asl, "caa", -DT / 3, c2[:])
                    mm(pa, asl, "caa", 2 * DT / 3, c3[:])
                    mm(pa, asl, "caa", -DT / 6, c4[:], stop=True)  # a1'
                    mm(B_S, sl, "ibd", 1.0, c4[:])
                    mm(B_Q, sl, "ibd", q4, c4[:])
                    # w update: w_{n+1} = w0 - (dt^2/12) Scum_{n+1} @ Cax
                    sc = spool.tile([128, NC2], F16, tag=f"sc{ch}")
                    nc.scalar.activation(sc[:], B_S[:, sl], COPY)
                    mm(pa, wsl, "cax", -D2 / 12, sc[:])
                    if st["sc_prev"] is not None:
                        mm(pa, wsl, "cax", D2 / 12, st["sc_prev"][:], stop=True)
                    st["sc_prev"] = sc
                    nwt = spool.tile([128, NC2], F16, tag=f"nw{ch}")
                    nc.scalar.activation(nwt[:], pa[:, wsl], COPY)
                    st["nw"] = nwt
                else:
                    mm(B_S, sl, "ibd", 1.0, c4[:], stop=True)
                yield

            def exit_chain(st):
                ch = st["ch"]
                sl = st["sl"]
                scf = epool.tile([128, NC2], F16, tag=f"scf{ch}")
                nc.scalar.activation(scf[:], B_S[:, sl], COPY)
                qcf = epool.tile([128, NC2], F16, tag=f"qcf{ch}")
                nc.scalar.activation(qcf[:], B_Q[:, sl], COPY)
                banks = [B_a[ch], B_b[ch], B_h[ch]]
                i = 0
                for th in range(2):
                    for tbl in range(2):
                        tb = th * 4 + 2 * ch + tbl
                        ov = opool.tile([128, DIM], F16, tag=f"ov{ch}")
                        ox = opool.tile([128, DIM], F16, tag=f"ox{ch}")
                        for dh in range(2):
                            dsl = slice(dh * NH, (dh + 1) * NH)
                            lhs_S = scf[th * 64 : (th + 1) * 64, tbl * 128 : (tbl + 1) * 128]
                            lhs_Q = qcf[th * 64 : (th + 1) * 64, tbl * 128 : (tbl + 1) * 128]
                            pv = banks[i % 3]
                            px = banks[(i + 1) % 3]
                            i += 2
                            # v half: S-gemm then DVE add of v0
                            nc.tensor.matmul(
                                pv[:],
                                lhs_S,
                                s_wcv[th * 64 : (th + 1) * 64, dsl],
                                start=True,
                                stop=True,
                                tile_position=(64 * th, 0),
                                skip_group_check=True,
                            )
                            nc.vector.tensor_add(ov[:, dsl], pv[:], s_vtok[:, tb, dsl])
                            # x half: (x0+v0) preloaded by identity matmul,
                            # Q-gemm accumulates, ACT materializes
                            nc.tensor.matmul(
                                px[:],
                                bdw("ibd", 1.0),
                                s_xtok[:, tb, dsl],
                                start=True,
                                stop=False,
                                skip_group_check=True,
                            )
                            nc.tensor.matmul(
                                px[:],
                                lhs_Q,
                                s_wcx[th * 64 : (th + 1) * 64, dsl],
                                start=False,
                                stop=True,
                                tile_position=(64 * th, 0),
                                skip_group_check=True,
                            )
                            nc.scalar.activation(ox[:, dsl], px[:], COPY)
                        nc.sync.dma_start(vout[tb * 128 : (tb + 1) * 128, :], ov[:])
                        nc.sync.dma_start(xout[tb * 128 : (tb + 1) * 128, :], ox[:])
                        yield

            chains = [
                {"ch": c, "sl": slice(c * NC2, (c + 1) * NC2), "nw": s_w0[c], "sc_prev": None}
                for c in range(2)
            ]

            def chain_gen(st):
                for n in range(STEPS):
                    yield from step_chain(n, st)
                yield from exit_chain(st)

            gens = [chain_gen(st) for st in chains]
            alive = True
            while alive:
                alive = False
                for g in gens:
                    try:
                        next(g)
                        alive = True
                    except StopIteration:
                        pass

    orig = nc.to_json_bytes
    nc.to_json_bytes = lambda: _split_waits(orig())
    _NC_CACHE = nc
    return nc


# -------------------------------------------------------------------- driver


def _run(x, v, Wa, Wb, Wx, Wc, trace=False):
    from concourse.bass_utils import run_bass_kernel_spmd

    x = np.asarray(x, np.float32).reshape(BATCH * SEQ, DIM)
    v = np.asarray(v, np.float32).reshape(BATCH * SEQ, DIM)
    consts = _host_consts(Wa, Wb, Wx, Wc)

    nc = _build_bass()
    in_maps = []
    for c in range(NCORES):
        xc = np.ascontiguousarray(x[c * TPC : (c + 1) * TPC])
        vc = np.ascontiguousarray(v[c * TPC : (c + 1) * TPC])
        m = {
            "xin": xc.astype(F16NP),
            "vin": vc.astype(F16NP),
            "xt": np.ascontiguousarray(xc.T).astype(F16NP),
            "vt": np.ascontiguousarray(vc.T).astype(F16NP),
        }
        m.update(consts)
        in_maps.append(m)

    res = run_bass_kernel_spmd(
        nc, in_maps, core_ids=list(range(NCORES)), trace=trace
    )
    xo = np.concatenate(
        [np.asarray(res.results[c]["xout"], np.float32) for c in range(NCORES)], axis=0
    )
    vo = np.concatenate(
        [np.asarray(res.results[c]["vout"], np.float32) for c in range(NCORES)], axis=0
    )
    return (xo.reshape(BATCH, SEQ, DIM), vo.reshape(BATCH, SEQ, DIM)), res


def kernel(x, v, Wa, Wb, Wx, Wc):
    (xo, vo), _ = _run(x, v, Wa, Wb, Wx, Wc, trace=False)
    return xo, vo


# revision 8
# speedup vs baseline: 1.4547x; 1.0482x over previous
"""Trainium2 Bass kernel for nn_AdjointManifoldBlock.

Reference computes 10 RK4 steps of:
    dx/dt = v ; dv/dt = -gamma,  gamma = ((v@Wa)*(v@Wb)*tanh(x@Wx)) @ Wc

Rank-space restructuring (per token, rank=64 state):
    a = v@Wa, b = v@Wb, h = x@Wx, w = (dt/2) v@Wx
    c_s = a_s * b_s * tanh(h_s)   per RK4 stage
    every stage update is a [64,64] GEMM with Caa=Wc@Wa, Cab=Wc@Wb, Cax=Wc@Wx
    x_T = x0 + v0 - (dt^2/6) Q @ Wc,  v_T = v0 - (dt/6) S @ Wc
    S = sum S_n, Q = sum [(9-n) S_n + P_n]

Differences vs the earlier version of this kernel:
  - inputs are shipped twice: token-major fp32 (exit adds) and host-transposed
    feature-major fp16 (entry GEMMs) -> no PE transposes, no ACT copies
  - all step-loop matmul operands are fp16 (1 cycle/row at any width)
  - the b-chain lives in PSUM like a (PE undo-accumulation), killing the
    per-stage DVE badd; both per-stage products run on DVE (GPSIMD has no
    PSUM port)
  - the w-chain is derived from the S accumulator: w_n = w0 - (dt^2/12)
    Scum_n @ Cax, tracked in PSUM via per-step delta matmuls from fp16
    snapshots of S (2 matmuls/step instead of 4)
  - exit: x0+v0 pre-added on GPSIMD (idle otherwise) during the steps;
    final adds on DVE straight from PSUM; fp32 x,v DMA'd during the steps

Layout per core (1024 tokens): partition dim = [halfA ranks 0:64 | halfB
ranks 64:128], halves = tokens 0:512 / 512:1024; NSPLIT=2 column chains
(256 cols each) interleaved stage-by-stage for cross-engine overlap.
"""

import json
import numpy as np
import ml_dtypes

DIM = 1024
RANK = 64
STEPS = 10
DT = 0.1
BATCH, SEQ = 4, 2048
NCORES = 8
TPC = (BATCH * SEQ) // NCORES  # tokens per core = 1024
NH = TPC // 2  # tokens per stacked half = 512
NCH = DIM // 128  # feature chunks = 8
NSPLIT = 2
NC2 = NH // NSPLIT  # columns per chain = 256

D2 = DT * DT

CAA_SC = [-DT / 2, DT / 2, -DT, -DT / 6, -DT / 3, 2 * DT / 3]
CAB_SC = list(CAA_SC)
CAX_SC = [-D2 / 4, D2 / 4, -D2 / 2, D2 / 3, -D2 / 6, -D2 / 12, D2 / 12]
IBD_SC = sorted(
    {1.0, 2.0}
    | {float(10 - n) for n in range(STEPS)}
    | {float(19 - 2 * n) for n in range(STEPS)}
    | {float(9 - n) for n in range(STEPS) if 9 - n > 0}
)
NV = len(CAA_SC) + len(CAB_SC) + len(CAX_SC) + len(IBD_SC)

F16NP = np.float16


def _vidx(kind, scale):
    if kind == "caa":
        return CAA_SC.index(scale)
    if kind == "cab":
        return len(CAA_SC) + CAB_SC.index(scale)
    if kind == "cax":
        return len(CAA_SC) + len(CAB_SC) + CAX_SC.index(scale)
    if kind == "ibd":
        return len(CAA_SC) + len(CAB_SC) + len(CAX_SC) + IBD_SC.index(float(scale))
    raise KeyError(kind)


# ---------------------------------------------------------------- host consts


def _host_consts(Wa, Wb, Wx, Wc):
    Wa64 = np.asarray(Wa, np.float64)
    Wb64 = np.asarray(Wb, np.float64)
    Wx64 = np.asarray(Wx, np.float64)
    Wc64 = np.asarray(Wc, np.float64)

    Caa = Wc64 @ Wa64  # [64, 64]; row index = contraction side
    Cab = Wc64 @ Wb64
    Cax = Wc64 @ Wx64
    I64 = np.eye(RANK)

    mats = (
        [Caa * s for s in CAA_SC]
        + [Cab * s for s in CAB_SC]
        + [Cax * s for s in CAX_SC]
        + [I64 * s for s in IBD_SC]
    )
    bd = np.zeros((NV, 128, 128), np.float64)
    for i, m in enumerate(mats):
        bd[i, 0:64, 0:64] = m
        bd[i, 64:128, 64:128] = m
    bd = np.ascontiguousarray(bd.transpose(1, 0, 2)).astype(F16NP)  # [128, NV, 128]

    stk = np.stack(
        [W.reshape(NCH, 128, RANK) for W in (Wa64, Wb64, Wx64, (DT / 2) * Wx64)]
    )  # [4, 8, 128, 64]
    wsa = np.ascontiguousarray(stk.transpose(2, 0, 1, 3).reshape(128, 4 * NCH, RANK)).astype(
        F16NP
    )
    wcv1 = -(DT / 6) * Wc64  # [64, 1024]
    wcx1 = -(D2 / 6) * Wc64
    wcv = np.concatenate([wcv1, wcv1], axis=0).astype(F16NP)  # [128, 1024] duplicated
    wcx = np.concatenate([wcx1, wcx1], axis=0).astype(F16NP)

    return {"bd": bd, "wsa": wsa, "wcv": wcv, "wcx": wcx}


# ----------------------------------------------------------- BIR wait postpass


def _split_waits(data: bytes) -> bytes:
    """This walrus build accepts only one inline sync wait per instruction;
    move excess waits onto NoOps inserted before the instruction (the
    engine sequencer processes them in order, so semantics are identical)."""
    bir = json.loads(data)
    for fn in bir["functions"]:
        for blk in fn["blocks"]:
            out = []
            k = 0
            for inst in blk["instructions"]:
                si = inst.get("sync_info")
                if si and len(si.get("on_wait", [])) > 1:
                    waits = si["on_wait"]
                    pre = []
                    while len(waits) > 1:
                        chunk, waits = waits[:1], waits[1:]
                        k += 1
                        pre.append(
                            {
                                "name": f"{inst['name']}-w{k}",
                                "opcode": "NoOp",
                                "engine": inst["engine"],
                                "ins": [],
                                "outs": [],
                                "sync_info": {"on_wait": chunk, "on_update": []},
                            }
                        )
                    si["on_wait"] = waits
                    out.extend(pre)
                out.append(inst)
            blk["instructions"] = out
    return json.dumps(bir).encode()


# ---------------------------------------------------------------- bass builder

_NC_CACHE = None


def _build_bass():
    global _NC_CACHE
    if _NC_CACHE is not None:
        return _NC_CACHE

    import concourse.bass as bass
    import concourse.tile as tile
    import concourse.mybir as mybir

    F32 = mybir.dt.float32
    F16 = mybir.dt.float16
    TANH = mybir.ActivationFunctionType.Tanh
    COPY = mybir.ActivationFunctionType.Copy

    nc = bass.Bass("TRN2", target_bir_lowering=False, debug=False, num_devices=1)

    xin = nc.dram_tensor("xin", [TPC, DIM], F16, kind="ExternalInput").ap()
    vin = nc.dram_tensor("vin", [TPC, DIM], F16, kind="ExternalInput").ap()
    xtr = nc.dram_tensor("xt", [DIM, TPC], F16, kind="ExternalInput").ap()
    vtr = nc.dram_tensor("vt", [DIM, TPC], F16, kind="ExternalInput").ap()
    bdm = nc.dram_tensor("bd", [128, NV, 128], F16, kind="ExternalInput").ap()
    wsa = nc.dram_tensor("wsa", [128, 4 * NCH, RANK], F16, kind="ExternalInput").ap()
    wcv = nc.dram_tensor("wcv", [128, DIM], F16, kind="ExternalInput").ap()
    wcx = nc.dram_tensor("wcx", [128, DIM], F16, kind="ExternalInput").ap()
    xout = nc.dram_tensor("xout", [TPC, DIM], F16, kind="ExternalOutput").ap()
    vout = nc.dram_tensor("vout", [TPC, DIM], F16, kind="ExternalOutput").ap()

    with tile.TileContext(nc) as tc:
        with (
            tc.tile_pool(name="consts", bufs=1) as consts,
            tc.tile_pool(name="tpool", bufs=6) as tpool,
            tc.tile_pool(name="mpool", bufs=4) as mpool,
            tc.tile_pool(name="cpool", bufs=6) as cpool,
            tc.tile_pool(name="spool", bufs=3) as spool,
            tc.tile_pool(name="epool", bufs=1) as epool,
            tc.tile_pool(name="opool", bufs=4) as opool,
            tc.tile_pool(name="ps", bufs=1, space="PSUM") as ps,
        ):
            # ---------------- tiles
            s_bd = consts.tile([128, NV, 128], F16, tag="bd")
            s_wsa = consts.tile([128, 4 * NCH, RANK], F16, tag="wsa")
            s_wcv = consts.tile([128, DIM], F16, tag="wcv")
            s_wcx = consts.tile([128, DIM], F16, tag="wcx")
            s_vt = consts.tile([128, NCH, TPC], F16, tag="vt")
            s_xt = consts.tile([128, NCH, TPC], F16, tag="xt")
            s_vtok = consts.tile([128, NCH, DIM], F16, tag="vtok")
            s_xtok = consts.tile([128, NCH, DIM], F16, tag="xtok")

            B_a = [ps.tile([128, 2 * NC2], F32, tag=f"Ba{c}", name=f"Ba{c}") for c in range(2)]
            B_b = [ps.tile([128, 2 * NC2], F32, tag=f"Bb{c}", name=f"Bb{c}") for c in range(2)]
            B_h = [ps.tile([128, 2 * NC2], F32, tag=f"Bh{c}", name=f"Bh{c}") for c in range(2)]
            B_S = ps.tile([128, NH], F32, tag="BS")
            B_Q = ps.tile([128, NH], F32, tag="BQ")

            asl = slice(0, NC2)  # a/b/h state columns within chain banks
            wsl = slice(NC2, 2 * NC2)  # w columns within B_a

            # memsets: a-banks fully (w region too); b/h state region only
            for c in range(2):
                nc.vector.memset(B_a[c][:], 0.0)
                nc.vector.memset(B_b[c][:, asl], 0.0)
                nc.vector.memset(B_h[c][:, asl], 0.0)
            nc.vector.memset(B_S[:], 0.0)
            nc.vector.memset(B_Q[:], 0.0)

            def bdw(kind, scale):
                return s_bd[:, _vidx(kind, scale), :]

            # ---------------- const + input DMAs (order = DMA device order:
            # wsa first, chain0 pieces, bd, chain1 pieces)
            nc.sync.dma_start(s_wsa[:], wsa[:])
            for ch in range(2):
                c0 = ch * NC2
                for hb in range(2):
                    t0 = hb * NH + c0
                    for src, dst in ((vtr, s_vt), (xtr, s_xt)):
                        sv = src.rearrange("(k p) c -> p k c", p=128)
                        nc.sync.dma_start(
                            dst[:, :, t0 : t0 + NC2], sv[:, :, t0 : t0 + NC2]
                        )
                if ch == 0:
                    nc.sync.dma_start(s_bd[:], bdm[:])
            for ch in range(2):
                c0 = ch * NC2
                for hb in range(2):
                    t0 = hb * NH + c0
                    for k in range(NCH):
                        last = k == NCH - 1
                        # (tsel, moving src, dest bank, dest cols)
                        for tsel, smov, bank, cols in (
                            (2, s_xt, B_h[ch], asl),  # h  (first: gates t1)
                            (1, s_vt, B_b[ch], asl),  # b  (gates m1)
                            (0, s_vt, B_a[ch], asl),  # a
                            (3, s_vt, B_a[ch], wsl),  # w0
                        ):
                            nc.tensor.matmul(
                                bank[hb * 64 : (hb + 1) * 64, cols],
                                s_wsa[:, tsel * NCH + k, :],
                                smov[:, k, t0 : t0 + NC2],
                                start=False,
                                stop=last,
                                tile_position=(0, 64 * hb) if hb else None,
                                skip_group_check=True,
                            )

            # w0 -> fp16 (serves as nw for step 0)
            s_w0 = []
            for ch in range(2):
                w0t = consts.tile([128, NC2], F16, tag=f"w0_{ch}")
                nc.scalar.activation(w0t[:], B_a[ch][:, wsl], COPY)
                s_w0.append(w0t)

            # ---------------- token-major fp32 inputs (needed only at exit)
            nc.sync.dma_start(
                s_vtok[:], vin.rearrange("(tb p) c -> p tb c", p=128)
            )
            nc.sync.dma_start(
                s_xtok[:], xin.rearrange("(tb p) c -> p tb c", p=128)
            )
            nc.sync.dma_start(s_wcv[:], wcv[:])
            nc.sync.dma_start(s_wcx[:], wcx[:])
            # x0 += v0 in place on GPSIMD (idle during the steps)
            for tb in range(NCH):
                nc.gpsimd.tensor_add(
                    s_xtok[:, tb, :], s_xtok[:, tb, :], s_vtok[:, tb, :]
                )

            # ---------------- the 10 RK4 steps
            def mm(bank, sl, kind, scale, rhs, stop=False):
                nc.tensor.matmul(
                    bank[:, sl],
                    bdw(kind, scale),
                    rhs,
                    start=False,
                    stop=stop,
                    skip_group_check=True,
                )

            def step_chain(n, st):
                ch = st["ch"]
                sl = st["sl"]  # chain's columns in B_S/B_Q
                pa, pb, ph = B_a[ch], B_b[ch], B_h[ch]
                last = n == STEPS - 1
                q1, q23, q4 = float(10 - n), float(19 - 2 * n), float(9 - n)
                nw = st["nw"]

                def tanh():
                    t = tpool.tile([128, NC2], F16, tag=f"t{ch}")
                    nc.scalar.activation(t[:], ph[:, asl], TANH)
                    return t

                def prod(b_src, t_s):
                    m = mpool.tile([128, NC2], F16, tag=f"m{ch}")
                    nc.vector.tensor_mul(m[:], b_src, t_s[:])
                    c = cpool.tile([128, NC2], F16, tag=f"c{ch}")
                    nc.vector.tensor_mul(c[:], pa[:, asl], m[:])
                    return c

                # stage 1
                t1 = st.pop("t1n", None)
                if t1 is None:
                    t1 = tanh()
                mm(ph, asl, "ibd", 1.0, nw[:], stop=True)  # h2
                t2 = tanh()
                c1 = prod(pb[:, asl], t1)
                mm(pb, asl, "cab", -DT / 2, c1[:], stop=True)  # b2
                mm(pa, asl, "caa", -DT / 2, c1[:], stop=True)  # a2
                mm(ph, asl, "cax", -D2 / 4, c1[:], stop=True)  # h3
                mm(B_S, sl, "ibd", 1.0, c1[:])
                mm(B_Q, sl, "ibd", q1, c1[:])
                yield

                # stage 2
                t3 = tanh()
                c2 = prod(pb[:, asl], t2)
                mm(pb, asl, "cab", DT / 2, c1[:])
                mm(pb, asl, "cab", -DT / 2, c2[:], stop=True)  # b3
                mm(pa, asl, "caa", DT / 2, c1[:])
                mm(pa, asl, "caa", -DT / 2, c2[:], stop=True)  # a3
                mm(ph, asl, "ibd", 1.0, nw[:])
                mm(ph, asl, "cax", D2 / 4, c1[:])
                mm(ph, asl, "cax", -D2 / 2, c2[:], stop=True)  # h4
                mm(B_S, sl, "ibd", 2.0, c2[:])
                mm(B_Q, sl, "ibd", q23, c2[:])
                yield

                # stage 3; h1' is computable here (h4 is dead once t4 is
                # read), pulling the next step's t1 off the step boundary
                t4 = tanh()
                c3 = prod(pb[:, asl], t3)
                mm(pb, asl, "cab", DT / 2, c2[:])
                mm(pb, asl, "cab", -DT, c3[:], stop=True)  # b4
                if not last:
                    mm(ph, asl, "cax", -D2 / 6, c1[:])
                    mm(ph, asl, "cax", D2 / 3, c2[:])
                    mm(ph, asl, "cax", -D2 / 6, c3[:], stop=True)  # h1'
                    st["t1n"] = tanh()  # tanh(h1') for next step
                mm(pa, asl, "caa", DT / 2, c2[:])
                mm(pa, asl, "caa", -DT, c3[:], stop=True)  # a4
                mm(B_S, sl, "ibd", 2.0, c3[:])
                mm(B_Q, sl, "ibd", q23, c3[:], stop=last)
                yield

                # stage 4
                c4 = prod(pb[:, asl], t4)
                if not last:
                    mm(pb, asl, "cab", -DT / 6, c1[:])
                    mm(pb, asl, "cab", -DT / 3, c2[:])
                    mm(pb, asl, "cab", 2 * DT / 3, c3[:])
                    mm(pb, asl, "cab", -DT / 6, c4[:], stop=True)  # b1'
                    mm(pa, asl, "caa", -DT / 6, c1[:])
                    mm(pa, asl, "caa", -DT / 3, c2[:])
                    mm(pa, asl, "caa", 2 * DT / 3, c3[:])
                    mm(pa, asl, "caa", -DT / 6, c4[:], stop=True)  # a1'
                    mm(B_S, sl, "ibd", 1.0, c4[:])
                    mm(B_Q, sl, "ibd", q4, c4[:])
                    # w update: w_{n+1} = w0 - (dt^2/12) Scum_{n+1} @ Cax
                    sc = spool.tile([128, NC2], F16, tag=f"sc{ch}")
                    nc.scalar.activation(sc[:], B_S[:, sl], COPY)
                    mm(pa, wsl, "cax", -D2 / 12, sc[:])
                    if st["sc_prev"] is not None:
                        mm(pa, wsl, "cax", D2 / 12, st["sc_prev"][:], stop=True)
                    st["sc_prev"] = sc
                    nwt = spool.tile([128, NC2], F16, tag=f"nw{ch}")
                    nc.scalar.activation(nwt[:], pa[:, wsl], COPY)
                    st["nw"] = nwt
                else:
                    mm(B_S, sl, "ibd", 1.0, c4[:], stop=True)
                yield

            def exit_chain(st):
                ch = st["ch"]
                sl = st["sl"]
                scf = epool.tile([128, NC2], F16, tag=f"scf{ch}")
                nc.scalar.activation(scf[:], B_S[:, sl], COPY)
                qcf = epool.tile([128, NC2], F16, tag=f"qcf{ch}")
                nc.scalar.activation(qcf[:], B_Q[:, sl], COPY)
                banks = [B_a[ch], B_b[ch], B_h[ch]]
                i = 0
                for th in range(2):
                    for tbl in range(2):
                        tb = th * 4 + 2 * ch + tbl
                        ov = opool.tile([128, DIM], F16, tag=f"ov{ch}")
                        ox = opool.tile([128, DIM], F16, tag=f"ox{ch}")
                        for dh in range(2):
                            dsl = slice(dh * NH, (dh + 1) * NH)
                            lhs_S = scf[th * 64 : (th + 1) * 64, tbl * 128 : (tbl + 1) * 128]
                            lhs_Q = qcf[th * 64 : (th + 1) * 64, tbl * 128 : (tbl + 1) * 128]
                            pv = banks[i % 3]
                            px = banks[(i + 1) % 3]
                            i += 2
                            # v half: S-gemm then DVE add of v0
                            nc.tensor.matmul(
                                pv[:],
                                lhs_S,
                                s_wcv[th * 64 : (th + 1) * 64, dsl],
                                start=True,
                                stop=True,
                                tile_position=(64 * th, 0),
                                skip_group_check=True,
                            )
                            nc.vector.tensor_add(ov[:, dsl], pv[:], s_vtok[:, tb, dsl])
                            # x half: (x0+v0) preloaded by identity matmul,
                            # Q-gemm accumulates, ACT materializes
                            nc.tensor.matmul(
                                px[:],
                                bdw("ibd", 1.0),
                                s_xtok[:, tb, dsl],
                                start=True,
                                stop=False,
                                skip_group_check=True,
                            )
                            nc.tensor.matmul(
                                px[:],
                                lhs_Q,
                                s_wcx[th * 64 : (th + 1) * 64, dsl],
                                start=False,
                                stop=True,
                                tile_position=(64 * th, 0),
                                skip_group_check=True,
                            )
                            nc.scalar.activation(ox[:, dsl], px[:], COPY)
                        nc.sync.dma_start(vout[tb * 128 : (tb + 1) * 128, :], ov[:])
                        nc.sync.dma_start(xout[tb * 128 : (tb + 1) * 128, :], ox[:])
                        yield

            chains = [
                {"ch": c, "sl": slice(c * NC2, (c + 1) * NC2), "nw": s_w0[c], "sc_prev": None}
                for c in range(2)
            ]

            def chain_gen(st):
                for n in range(STEPS):
                    yield from step_chain(n, st)
                yield from exit_chain(st)

            gens = [chain_gen(st) for st in chains]
            alive = True
            while alive:
                alive = False
                for g in gens:
                    try:
                        next(g)
                        alive = True
                    except StopIteration:
                        pass

    orig = nc.to_json_bytes
    nc.to_json_bytes = lambda: _split_waits(orig())
    _NC_CACHE = nc
    return nc


# -------------------------------------------------------------------- driver


def _run(x, v, Wa, Wb, Wx, Wc, trace=False):
    from concourse.bass_utils import run_bass_kernel_spmd

    x = np.asarray(x, np.float32).reshape(BATCH * SEQ, DIM)
    v = np.asarray(v, np.float32).reshape(BATCH * SEQ, DIM)
    consts = _host_consts(Wa, Wb, Wx, Wc)

    nc = _build_bass()
    in_maps = []
    for c in range(NCORES):
        xc = np.ascontiguousarray(x[c * TPC : (c + 1) * TPC])
        vc = np.ascontiguousarray(v[c * TPC : (c + 1) * TPC])
        m = {
            "xin": xc.astype(F16NP),
            "vin": vc.astype(F16NP),
            "xt": np.ascontiguousarray(xc.T).astype(F16NP),
            "vt": np.ascontiguousarray(vc.T).astype(F16NP),
        }
        m.update(consts)
        in_maps.append(m)

    res = run_bass_kernel_spmd(
        nc, in_maps, core_ids=list(range(NCORES)), trace=trace
    )
    xo = np.concatenate(
        [np.asarray(res.results[c]["xout"], np.float32) for c in range(NCORES)], axis=0
    )
    vo = np.concatenate(
        [np.asarray(res.results[c]["vout"], np.float32) for c in range(NCORES)], axis=0
    )
    return (xo.reshape(BATCH, SEQ, DIM), vo.reshape(BATCH, SEQ, DIM)), res


def kernel(x, v, Wa, Wb, Wx, Wc):
    (xo, vo), _ = _run(x, v, Wa, Wb, Wx, Wc, trace=False)
    return xo, vo


# revision 9
# speedup vs baseline: 1.4724x; 1.0122x over previous
"""Trainium2 Bass kernel for nn_AdjointManifoldBlock.

Reference computes 10 RK4 steps of:
    dx/dt = v ; dv/dt = -gamma,  gamma = ((v@Wa)*(v@Wb)*tanh(x@Wx)) @ Wc

Rank-space restructuring (per token, rank=64 state):
    a = v@Wa, b = v@Wb, h = x@Wx, w = (dt/2) v@Wx
    c_s = a_s * b_s * tanh(h_s)   per RK4 stage
    every stage update is a [64,64] GEMM with Caa=Wc@Wa, Cab=Wc@Wb, Cax=Wc@Wx
    x_T = x0 + v0 - (dt^2/6) Q @ Wc,  v_T = v0 - (dt/6) S @ Wc
    S = sum S_n, Q = sum [(9-n) S_n + P_n]

Differences vs the earlier version of this kernel:
  - inputs are shipped twice: token-major fp32 (exit adds) and host-transposed
    feature-major fp16 (entry GEMMs) -> no PE transposes, no ACT copies
  - all step-loop matmul operands are fp16 (1 cycle/row at any width)
  - the b-chain lives in PSUM like a (PE undo-accumulation), killing the
    per-stage DVE badd; both per-stage products run on DVE (GPSIMD has no
    PSUM port)
  - the w-chain is derived from the S accumulator: w_n = w0 - (dt^2/12)
    Scum_n @ Cax, tracked in PSUM via per-step delta matmuls from fp16
    snapshots of S (2 matmuls/step instead of 4)
  - exit: x0+v0 pre-added on GPSIMD (idle otherwise) during the steps;
    final adds on DVE straight from PSUM; fp32 x,v DMA'd during the steps

Layout per core (1024 tokens): partition dim = [halfA ranks 0:64 | halfB
ranks 64:128], halves = tokens 0:512 / 512:1024; NSPLIT=2 column chains
(256 cols each) interleaved stage-by-stage for cross-engine overlap.
"""

import json
import numpy as np
import ml_dtypes

DIM = 1024
RANK = 64
STEPS = 10
DT = 0.1
BATCH, SEQ = 4, 2048
NCORES = 8
TPC = (BATCH * SEQ) // NCORES  # tokens per core = 1024
NH = TPC // 2  # tokens per stacked half = 512
NCH = DIM // 128  # feature chunks = 8
NSPLIT = 2
NC2 = NH // NSPLIT  # columns per chain = 256

D2 = DT * DT

CAA_SC = [-DT / 2, DT / 2, -DT, -DT / 6, -DT / 3, 2 * DT / 3]
CAB_SC = list(CAA_SC)
CAX_SC = [-D2 / 4, D2 / 4, -D2 / 2, D2 / 3, -D2 / 6, -D2 / 12, D2 / 12]
IBD_SC = sorted(
    {1.0, 2.0}
    | {float(10 - n) for n in range(STEPS)}
    | {float(19 - 2 * n) for n in range(STEPS)}
    | {float(9 - n) for n in range(STEPS) if 9 - n > 0}
)
NV = len(CAA_SC) + len(CAB_SC) + len(CAX_SC) + len(IBD_SC)

F16NP = np.float16


def _vidx(kind, scale):
    if kind == "caa":
        return CAA_SC.index(scale)
    if kind == "cab":
        return len(CAA_SC) + CAB_SC.index(scale)
    if kind == "cax":
        return len(CAA_SC) + len(CAB_SC) + CAX_SC.index(scale)
    if kind == "ibd":
        return len(CAA_SC) + len(CAB_SC) + len(CAX_SC) + IBD_SC.index(float(scale))
    raise KeyError(kind)


# ---------------------------------------------------------------- host consts


def _host_consts(Wa, Wb, Wx, Wc):
    Wa64 = np.asarray(Wa, np.float64)
    Wb64 = np.asarray(Wb, np.float64)
    Wx64 = np.asarray(Wx, np.float64)
    Wc64 = np.asarray(Wc, np.float64)

    Caa = Wc64 @ Wa64  # [64, 64]; row index = contraction side
    Cab = Wc64 @ Wb64
    Cax = Wc64 @ Wx64
    I64 = np.eye(RANK)

    mats = (
        [Caa * s for s in CAA_SC]
        + [Cab * s for s in CAB_SC]
        + [Cax * s for s in CAX_SC]
        + [I64 * s for s in IBD_SC]
    )
    bd = np.zeros((NV, 128, 128), np.float64)
    for i, m in enumerate(mats):
        bd[i, 0:64, 0:64] = m
        bd[i, 64:128, 64:128] = m
    bd = np.ascontiguousarray(bd.transpose(1, 0, 2)).astype(F16NP)  # [128, NV, 128]

    stk = np.stack(
        [W.reshape(NCH, 128, RANK) for W in (Wa64, Wb64, Wx64, (DT / 2) * Wx64)]
    )  # [4, 8, 128, 64]
    wsa = np.ascontiguousarray(stk.transpose(2, 0, 1, 3).reshape(128, 4 * NCH, RANK)).astype(
        F16NP
    )
    wcv1 = -(DT / 6) * Wc64  # [64, 1024]
    wcx1 = -(D2 / 6) * Wc64
    wcv = np.concatenate([wcv1, wcv1], axis=0).astype(F16NP)  # [128, 1024] duplicated
    wcx = np.concatenate([wcx1, wcx1], axis=0).astype(F16NP)

    return {"bd": bd, "wsa": wsa, "wcv": wcv, "wcx": wcx}


# ----------------------------------------------------------- BIR wait postpass


def _split_waits(data: bytes) -> bytes:
    """This walrus build accepts only one inline sync wait per instruction;
    move excess waits onto NoOps inserted before the instruction (the
    engine sequencer processes them in order, so semantics are identical)."""
    bir = json.loads(data)
    for fn in bir["functions"]:
        for blk in fn["blocks"]:
            out = []
            k = 0
            for inst in blk["instructions"]:
                si = inst.get("sync_info")
                if si and len(si.get("on_wait", [])) > 1:
                    waits = si["on_wait"]
                    pre = []
                    while len(waits) > 1:
                        chunk, waits = waits[:1], waits[1:]
                        k += 1
                        pre.append(
                            {
                                "name": f"{inst['name']}-w{k}",
                                "opcode": "NoOp",
                                "engine": inst["engine"],
                                "ins": [],
                                "outs": [],
                                "sync_info": {"on_wait": chunk, "on_update": []},
                            }
                        )
                    si["on_wait"] = waits
                    out.extend(pre)
                out.append(inst)
            blk["instructions"] = out
    return json.dumps(bir).encode()


# ---------------------------------------------------------------- bass builder

_NC_CACHE = None


def _build_bass():
    global _NC_CACHE
    if _NC_CACHE is not None:
        return _NC_CACHE

    import concourse.bass as bass
    import concourse.tile as tile
    import concourse.mybir as mybir

    F32 = mybir.dt.float32
    F16 = mybir.dt.float16
    TANH = mybir.ActivationFunctionType.Tanh
    COPY = mybir.ActivationFunctionType.Copy

    nc = bass.Bass("TRN2", target_bir_lowering=False, debug=False, num_devices=1)

    xin = nc.dram_tensor("xin", [TPC, DIM], F16, kind="ExternalInput").ap()
    vin = nc.dram_tensor("vin", [TPC, DIM], F16, kind="ExternalInput").ap()
    xtr = nc.dram_tensor("xt", [DIM, TPC], F16, kind="ExternalInput").ap()
    vtr = nc.dram_tensor("vt", [DIM, TPC], F16, kind="ExternalInput").ap()
    bdm = nc.dram_tensor("bd", [128, NV, 128], F16, kind="ExternalInput").ap()
    wsa = nc.dram_tensor("wsa", [128, 4 * NCH, RANK], F16, kind="ExternalInput").ap()
    wcv = nc.dram_tensor("wcv", [128, DIM], F16, kind="ExternalInput").ap()
    wcx = nc.dram_tensor("wcx", [128, DIM], F16, kind="ExternalInput").ap()
    xout = nc.dram_tensor("xout", [TPC, DIM], F16, kind="ExternalOutput").ap()
    vout = nc.dram_tensor("vout", [TPC, DIM], F16, kind="ExternalOutput").ap()

    with tile.TileContext(nc) as tc:
        with (
            tc.tile_pool(name="consts", bufs=1) as consts,
            tc.tile_pool(name="tpool", bufs=6) as tpool,
            tc.tile_pool(name="mpool", bufs=4) as mpool,
            tc.tile_pool(name="cpool", bufs=6) as cpool,
            tc.tile_pool(name="spool", bufs=3) as spool,
            tc.tile_pool(name="epool", bufs=1) as epool,
            tc.tile_pool(name="opool", bufs=4) as opool,
            tc.tile_pool(name="ps", bufs=1, space="PSUM") as ps,
        ):
            # ---------------- tiles
            s_bd = consts.tile([128, NV, 128], F16, tag="bd")
            s_wsa = consts.tile([128, 4 * NCH, RANK], F16, tag="wsa")
            s_wcv = consts.tile([128, DIM], F16, tag="wcv")
            s_wcx = consts.tile([128, DIM], F16, tag="wcx")
            s_vt = consts.tile([128, NCH, TPC], F16, tag="vt")
            s_xt = consts.tile([128, NCH, TPC], F16, tag="xt")
            s_vtok = consts.tile([128, NCH, DIM], F16, tag="vtok")
            s_xtok = consts.tile([128, NCH, DIM], F16, tag="xtok")

            B_a = [ps.tile([128, 2 * NC2], F32, tag=f"Ba{c}", name=f"Ba{c}") for c in range(2)]
            B_b = [ps.tile([128, 2 * NC2], F32, tag=f"Bb{c}", name=f"Bb{c}") for c in range(2)]
            B_h = [ps.tile([128, 2 * NC2], F32, tag=f"Bh{c}", name=f"Bh{c}") for c in range(2)]
            B_S = ps.tile([128, NH], F32, tag="BS")
            B_Q = ps.tile([128, NH], F32, tag="BQ")

            asl = slice(0, NC2)  # a/b/h state columns within chain banks
            wsl = slice(NC2, 2 * NC2)  # w columns within B_a

            # memsets: a-banks fully (w region too); b/h state region only
            for c in range(2):
                nc.vector.memset(B_a[c][:], 0.0)
                nc.vector.memset(B_b[c][:, asl], 0.0)
                nc.vector.memset(B_h[c][:, asl], 0.0)
            nc.vector.memset(B_S[:], 0.0)
            nc.vector.memset(B_Q[:], 0.0)

            def bdw(kind, scale):
                return s_bd[:, _vidx(kind, scale), :]

            # ---------------- const + input DMAs (order = DMA device order:
            # wsa first, chain0 pieces, bd, chain1 pieces)
            nc.sync.dma_start(s_wsa[:], wsa[:])
            for ch in range(2):
                c0 = ch * NC2
                for hb in range(2):
                    t0 = hb * NH + c0
                    for src, dst in ((vtr, s_vt), (xtr, s_xt)):
                        sv = src.rearrange("(k p) c -> p k c", p=128)
                        nc.sync.dma_start(
                            dst[:, :, t0 : t0 + NC2], sv[:, :, t0 : t0 + NC2]
                        )
                if ch == 0:
                    nc.sync.dma_start(s_bd[:], bdm[:])
            for ch in range(2):
                c0 = ch * NC2
                for hb in range(2):
                    t0 = hb * NH + c0
                    for k in range(NCH):
                        last = k == NCH - 1
                        # (tsel, moving src, dest bank, dest cols)
                        for tsel, smov, bank, cols in (
                            (2, s_xt, B_h[ch], asl),  # h  (first: gates t1)
                            (1, s_vt, B_b[ch], asl),  # b  (gates m1)
                            (0, s_vt, B_a[ch], asl),  # a
                            (3, s_vt, B_a[ch], wsl),  # w0
                        ):
                            nc.tensor.matmul(
                                bank[hb * 64 : (hb + 1) * 64, cols],
                                s_wsa[:, tsel * NCH + k, :],
                                smov[:, k, t0 : t0 + NC2],
                                start=False,
                                stop=last,
                                tile_position=(0, 64 * hb) if hb else None,
                                skip_group_check=True,
                            )

            # w0 -> fp16 (serves as nw for step 0)
            s_w0 = []
            for ch in range(2):
                w0t = consts.tile([128, NC2], F16, tag=f"w0_{ch}")
                nc.scalar.activation(w0t[:], B_a[ch][:, wsl], COPY)
                s_w0.append(w0t)

            # ---------------- token-major fp32 inputs (needed only at exit)
            nc.sync.dma_start(
                s_vtok[:], vin.rearrange("(tb p) c -> p tb c", p=128)
            )
            nc.sync.dma_start(
                s_xtok[:], xin.rearrange("(tb p) c -> p tb c", p=128)
            )
            nc.sync.dma_start(s_wcv[:], wcv[:])
            nc.sync.dma_start(s_wcx[:], wcx[:])
            # x0 += v0 in place on GPSIMD (idle during the steps)
            for tb in range(NCH):
                nc.gpsimd.tensor_add(
                    s_xtok[:, tb, :], s_xtok[:, tb, :], s_vtok[:, tb, :]
                )

            # ---------------- the 10 RK4 steps
            def mm(bank, sl, kind, scale, rhs, stop=False):
                nc.tensor.matmul(
                    bank[:, sl],
                    bdw(kind, scale),
                    rhs,
                    start=False,
                    stop=stop,
                    skip_group_check=True,
                )

            def step_chain(n, st):
                ch = st["ch"]
                sl = st["sl"]  # chain's columns in B_S/B_Q
                pa, pb, ph = B_a[ch], B_b[ch], B_h[ch]
                last = n == STEPS - 1
                q1, q23, q4 = float(10 - n), float(19 - 2 * n), float(9 - n)
                nw = st["nw"]

                def tanh():
                    t = tpool.tile([128, NC2], F16, tag=f"t{ch}")
                    nc.scalar.activation(t[:], ph[:, asl], TANH)
                    return t

                def prod(b_src, t_s):
                    m = mpool.tile([128, NC2], F16, tag=f"m{ch}")
                    nc.vector.tensor_mul(m[:], b_src, t_s[:])
                    c = cpool.tile([128, NC2], F16, tag=f"c{ch}")
                    nc.vector.tensor_mul(c[:], pa[:, asl], m[:])
                    return c

                # stage 1
                t1 = st.pop("t1n", None)
                if t1 is None:
                    t1 = tanh()
                mm(ph, asl, "ibd", 1.0, nw[:], stop=True)  # h2
                t2 = tanh()
                c1 = prod(pb[:, asl], t1)
                mm(pb, asl, "cab", -DT / 2, c1[:], stop=True)  # b2
                mm(pa, asl, "caa", -DT / 2, c1[:], stop=True)  # a2
                mm(ph, asl, "cax", -D2 / 4, c1[:], stop=True)  # h3
                mm(B_S, sl, "ibd", 1.0, c1[:])
                mm(B_Q, sl, "ibd", q1, c1[:])
                yield

                # stage 2
                t3 = tanh()
                c2 = prod(pb[:, asl], t2)
                mm(pb, asl, "cab", DT / 2, c1[:])
                mm(pb, asl, "cab", -DT / 2, c2[:], stop=True)  # b3
                mm(pa, asl, "caa", DT / 2, c1[:])
                mm(pa, asl, "caa", -DT / 2, c2[:], stop=True)  # a3
                mm(ph, asl, "ibd", 1.0, nw[:])
                mm(ph, asl, "cax", D2 / 4, c1[:])
                mm(ph, asl, "cax", -D2 / 2, c2[:], stop=True)  # h4
                mm(B_S, sl, "ibd", 2.0, c2[:])
                mm(B_Q, sl, "ibd", q23, c2[:])
                yield

                # stage 3; h1' is computable here (h4 is dead once t4 is
                # read), pulling the next step's t1 off the step boundary
                t4 = tanh()
                c3 = prod(pb[:, asl], t3)
                mm(pb, asl, "cab", DT / 2, c2[:])
                mm(pb, asl, "cab", -DT, c3[:], stop=True)  # b4
                if not last:
                    mm(ph, asl, "cax", -D2 / 6, c1[:])
                    mm(ph, asl, "cax", D2 / 3, c2[:])
                    mm(ph, asl, "cax", -D2 / 6, c3[:], stop=True)  # h1'
                    st["t1n"] = tanh()  # tanh(h1') for next step
                mm(pa, asl, "caa", DT / 2, c2[:])
                mm(pa, asl, "caa", -DT, c3[:], stop=True)  # a4
                mm(B_S, sl, "ibd", 2.0, c3[:])
                mm(B_Q, sl, "ibd", q23, c3[:], stop=(last and ch == 1))
                yield

                # stage 4
                c4 = prod(pb[:, asl], t4)
                if not last:
                    mm(pb, asl, "cab", -DT / 6, c1[:])
                    mm(pb, asl, "cab", -DT / 3, c2[:])
                    mm(pb, asl, "cab", 2 * DT / 3, c3[:])
                    mm(pb, asl, "cab", -DT / 6, c4[:], stop=True)  # b1'
                    mm(pa, asl, "caa", -DT / 6, c1[:])
                    mm(pa, asl, "caa", -DT / 3, c2[:])
                    mm(pa, asl, "caa", 2 * DT / 3, c3[:])
                    mm(pa, asl, "caa", -DT / 6, c4[:], stop=True)  # a1'
                    mm(B_S, sl, "ibd", 1.0, c4[:])
                    mm(B_Q, sl, "ibd", q4, c4[:])
                    # w update: w_{n+1} = w0 - (dt^2/12) Scum_{n+1} @ Cax
                    sc = spool.tile([128, NC2], F16, tag=f"sc{ch}")
                    nc.scalar.activation(sc[:], B_S[:, sl], COPY)
                    mm(pa, wsl, "cax", -D2 / 12, sc[:])
                    if st["sc_prev"] is not None:
                        mm(pa, wsl, "cax", D2 / 12, st["sc_prev"][:], stop=True)
                    st["sc_prev"] = sc
                    nwt = spool.tile([128, NC2], F16, tag=f"nw{ch}")
                    nc.scalar.activation(nwt[:], pa[:, wsl], COPY)
                    st["nw"] = nwt
                else:
                    mm(B_S, sl, "ibd", 1.0, c4[:], stop=(ch == 1))
                yield

            def exit_chain(st):
                ch = st["ch"]
                sl = st["sl"]
                scf = epool.tile([128, NC2], F16, tag=f"scf{ch}")
                nc.scalar.activation(scf[:], B_S[:, sl], COPY)
                qcf = epool.tile([128, NC2], F16, tag=f"qcf{ch}")
                nc.scalar.activation(qcf[:], B_Q[:, sl], COPY)
                banks = [B_a[ch], B_b[ch], B_h[ch]]
                i = 0
                for th in range(2):
                    for tbl in range(2):
                        tb = th * 4 + 2 * ch + tbl
                        ov = opool.tile([128, DIM], F16, tag=f"ov{ch}")
                        ox = opool.tile([128, DIM], F16, tag=f"ox{ch}")
                        for dh in range(2):
                            dsl = slice(dh * NH, (dh + 1) * NH)
                            lhs_S = scf[th * 64 : (th + 1) * 64, tbl * 128 : (tbl + 1) * 128]
                            lhs_Q = qcf[th * 64 : (th + 1) * 64, tbl * 128 : (tbl + 1) * 128]
                            pv = banks[i % 3]
                            px = banks[(i + 1) % 3]
                            i += 2
                            # v half: S-gemm then DVE add of v0
                            nc.tensor.matmul(
                                pv[:],
                                lhs_S,
                                s_wcv[th * 64 : (th + 1) * 64, dsl],
                                start=True,
                                stop=True,
                                tile_position=(64 * th, 0),
                                skip_group_check=True,
                            )
                            nc.vector.tensor_add(ov[:, dsl], pv[:], s_vtok[:, tb, dsl])
                            # x half: (x0+v0) preloaded by identity matmul,
                            # Q-gemm accumulates, ACT materializes
                            nc.tensor.matmul(
                                px[:],
                                bdw("ibd", 1.0),
                                s_xtok[:, tb, dsl],
                                start=True,
                                stop=False,
                                skip_group_check=True,
                            )
                            nc.tensor.matmul(
                                px[:],
                                lhs_Q,
                                s_wcx[th * 64 : (th + 1) * 64, dsl],
                                start=False,
                                stop=True,
                                tile_position=(64 * th, 0),
                                skip_group_check=True,
                            )
                            nc.scalar.activation(ox[:, dsl], px[:], COPY)
                        nc.sync.dma_start(vout[tb * 128 : (tb + 1) * 128, :], ov[:])
                        nc.sync.dma_start(xout[tb * 128 : (tb + 1) * 128, :], ox[:])
                        yield

            chains = [
                {"ch": c, "sl": slice(c * NC2, (c + 1) * NC2), "nw": s_w0[c], "sc_prev": None}
                for c in range(2)
            ]

            def chain_gen(st):
                for n in range(STEPS):
                    yield from step_chain(n, st)
                yield from exit_chain(st)

            gens = [chain_gen(st) for st in chains]
            # stagger: chain0 two stages ahead so stage-4 PE bursts interleave
            next(gens[0])
            next(gens[0])
            alive = True
            while alive:
                alive = False
                for g in gens:
                    try:
                        next(g)
                        alive = True
                    except StopIteration:
                        pass

    orig = nc.to_json_bytes
    nc.to_json_bytes = lambda: _split_waits(orig())
    _NC_CACHE = nc
    return nc


# -------------------------------------------------------------------- driver


def _run(x, v, Wa, Wb, Wx, Wc, trace=False):
    from concourse.bass_utils import run_bass_kernel_spmd

    x = np.asarray(x, np.float32).reshape(BATCH * SEQ, DIM)
    v = np.asarray(v, np.float32).reshape(BATCH * SEQ, DIM)
    consts = _host_consts(Wa, Wb, Wx, Wc)

    nc = _build_bass()
    in_maps = []
    for c in range(NCORES):
        xc = np.ascontiguousarray(x[c * TPC : (c + 1) * TPC])
        vc = np.ascontiguousarray(v[c * TPC : (c + 1) * TPC])
        m = {
            "xin": xc.astype(F16NP),
            "vin": vc.astype(F16NP),
            "xt": np.ascontiguousarray(xc.T).astype(F16NP),
            "vt": np.ascontiguousarray(vc.T).astype(F16NP),
        }
        m.update(consts)
        in_maps.append(m)

    res = run_bass_kernel_spmd(
        nc, in_maps, core_ids=list(range(NCORES)), trace=trace
    )
    xo = np.concatenate(
        [np.asarray(res.results[c]["xout"], np.float32) for c in range(NCORES)], axis=0
    )
    vo = np.concatenate(
        [np.asarray(res.results[c]["vout"], np.float32) for c in range(NCORES)], axis=0
    )
    return (xo.reshape(BATCH, SEQ, DIM), vo.reshape(BATCH, SEQ, DIM)), res


def kernel(x, v, Wa, Wb, Wx, Wc):
    (xo, vo), _ = _run(x, v, Wa, Wb, Wx, Wc, trace=False)
    return xo, vo


# revision 12
# speedup vs baseline: 1.4925x; 1.0137x over previous
"""Trainium2 Bass kernel for nn_AdjointManifoldBlock.

Reference computes 10 RK4 steps of:
    dx/dt = v ; dv/dt = -gamma,  gamma = ((v@Wa)*(v@Wb)*tanh(x@Wx)) @ Wc

Rank-space restructuring (per token, rank=64 state):
    a = v@Wa, b = v@Wb, h = x@Wx, w = (dt/2) v@Wx
    c_s = a_s * b_s * tanh(h_s)   per RK4 stage
    every stage update is a [64,64] GEMM with Caa=Wc@Wa, Cab=Wc@Wb, Cax=Wc@Wx
    x_T = x0 + v0 - (dt^2/6) Q @ Wc,  v_T = v0 - (dt/6) S @ Wc
    S = sum S_n, Q = sum [(9-n) S_n + P_n]

Differences vs the earlier version of this kernel:
  - inputs are shipped twice: token-major fp32 (exit adds) and host-transposed
    feature-major fp16 (entry GEMMs) -> no PE transposes, no ACT copies
  - all step-loop matmul operands are fp16 (1 cycle/row at any width)
  - the b-chain lives in PSUM like a (PE undo-accumulation), killing the
    per-stage DVE badd; both per-stage products run on DVE (GPSIMD has no
    PSUM port)
  - the w-chain is derived from the S accumulator: w_n = w0 - (dt^2/12)
    Scum_n @ Cax, tracked in PSUM via per-step delta matmuls from fp16
    snapshots of S (2 matmuls/step instead of 4)
  - exit: x0+v0 pre-added on GPSIMD (idle otherwise) during the steps;
    final adds on DVE straight from PSUM; fp32 x,v DMA'd during the steps

Layout per core (1024 tokens): partition dim = [halfA ranks 0:64 | halfB
ranks 64:128], halves = tokens 0:512 / 512:1024; NSPLIT=2 column chains
(256 cols each) interleaved stage-by-stage for cross-engine overlap.
"""

import json
import numpy as np
import ml_dtypes

DIM = 1024
RANK = 64
STEPS = 10
DT = 0.1
BATCH, SEQ = 4, 2048
NCORES = 8
TPC = (BATCH * SEQ) // NCORES  # tokens per core = 1024
NH = TPC // 2  # tokens per stacked half = 512
NCH = DIM // 128  # feature chunks = 8
NSPLIT = 2
NC2 = NH // NSPLIT  # columns per chain = 256

D2 = DT * DT

CAA_SC = [-DT / 2, DT / 2, -DT, -DT / 6, -DT / 3, 2 * DT / 3]
CAB_SC = list(CAA_SC)
CAX_SC = [-D2 / 4, D2 / 4, -D2 / 2, D2 / 3, -D2 / 6, -D2 / 12, D2 / 12]
IBD_SC = sorted(
    {1.0, 2.0}
    | {float(10 - n) for n in range(STEPS)}
    | {float(19 - 2 * n) for n in range(STEPS)}
    | {float(9 - n) for n in range(STEPS) if 9 - n > 0}
)
NV = len(CAA_SC) + len(CAB_SC) + len(CAX_SC) + len(IBD_SC)

F16NP = np.float16


def _vidx(kind, scale):
    if kind == "caa":
        return CAA_SC.index(scale)
    if kind == "cab":
        return len(CAA_SC) + CAB_SC.index(scale)
    if kind == "cax":
        return len(CAA_SC) + len(CAB_SC) + CAX_SC.index(scale)
    if kind == "ibd":
        return len(CAA_SC) + len(CAB_SC) + len(CAX_SC) + IBD_SC.index(float(scale))
    raise KeyError(kind)


# ---------------------------------------------------------------- host consts


def _host_consts(Wa, Wb, Wx, Wc):
    Wa64 = np.asarray(Wa, np.float64)
    Wb64 = np.asarray(Wb, np.float64)
    Wx64 = np.asarray(Wx, np.float64)
    Wc64 = np.asarray(Wc, np.float64)

    Caa = Wc64 @ Wa64  # [64, 64]; row index = contraction side
    Cab = Wc64 @ Wb64
    Cax = Wc64 @ Wx64
    I64 = np.eye(RANK)

    mats = (
        [Caa * s for s in CAA_SC]
        + [Cab * s for s in CAB_SC]
        + [Cax * s for s in CAX_SC]
        + [I64 * s for s in IBD_SC]
    )
    bd = np.zeros((NV, 128, 128), np.float64)
    for i, m in enumerate(mats):
        bd[i, 0:64, 0:64] = m
        bd[i, 64:128, 64:128] = m
    bd = np.ascontiguousarray(bd.transpose(1, 0, 2)).astype(F16NP)  # [128, NV, 128]

    stk = np.stack(
        [W.reshape(NCH, 128, RANK) for W in (Wa64, Wb64, Wx64, (DT / 2) * Wx64)]
    )  # [4, 8, 128, 64]
    wsa = np.ascontiguousarray(stk.transpose(2, 0, 1, 3).reshape(128, 4 * NCH, RANK)).astype(
        F16NP
    )
    wcv1 = -(DT / 6) * Wc64  # [64, 1024]
    wcx1 = -(D2 / 6) * Wc64
    wcv = np.concatenate([wcv1, wcv1], axis=0).astype(F16NP)  # [128, 1024] duplicated
    wcx = np.concatenate([wcx1, wcx1], axis=0).astype(F16NP)

    return {"bd": bd, "wsa": wsa, "wcv": wcv, "wcx": wcx}


# ----------------------------------------------------------- BIR wait postpass


def _split_waits(data: bytes) -> bytes:
    """This walrus build accepts only one inline sync wait per instruction;
    move excess waits onto NoOps inserted before the instruction (the
    engine sequencer processes them in order, so semantics are identical)."""
    bir = json.loads(data)
    for fn in bir["functions"]:
        for blk in fn["blocks"]:
            out = []
            k = 0
            for inst in blk["instructions"]:
                si = inst.get("sync_info")
                if si and len(si.get("on_wait", [])) > 1:
                    waits = si["on_wait"]
                    pre = []
                    while len(waits) > 1:
                        chunk, waits = waits[:1], waits[1:]
                        k += 1
                        pre.append(
                            {
                                "name": f"{inst['name']}-w{k}",
                                "opcode": "NoOp",
                                "engine": inst["engine"],
                                "ins": [],
                                "outs": [],
                                "sync_info": {"on_wait": chunk, "on_update": []},
                            }
                        )
                    si["on_wait"] = waits
                    out.extend(pre)
                out.append(inst)
            blk["instructions"] = out
    return json.dumps(bir).encode()


# ---------------------------------------------------------------- bass builder

_NC_CACHE = None


def _build_bass():
    global _NC_CACHE
    if _NC_CACHE is not None:
        return _NC_CACHE

    import concourse.bass as bass
    import concourse.tile as tile
    import concourse.mybir as mybir

    F32 = mybir.dt.float32
    F16 = mybir.dt.float16
    TANH = mybir.ActivationFunctionType.Tanh
    COPY = mybir.ActivationFunctionType.Copy

    nc = bass.Bass("TRN2", target_bir_lowering=False, debug=False, num_devices=1)

    xin = nc.dram_tensor("xin", [TPC, DIM], F16, kind="ExternalInput").ap()
    vin = nc.dram_tensor("vin", [TPC, DIM], F16, kind="ExternalInput").ap()
    xtr = nc.dram_tensor("xt", [DIM, TPC], F16, kind="ExternalInput").ap()
    vtr = nc.dram_tensor("vt", [DIM, TPC], F16, kind="ExternalInput").ap()
    bdm = nc.dram_tensor("bd", [128, NV, 128], F16, kind="ExternalInput").ap()
    wsa = nc.dram_tensor("wsa", [128, 4 * NCH, RANK], F16, kind="ExternalInput").ap()
    wcv = nc.dram_tensor("wcv", [128, DIM], F16, kind="ExternalInput").ap()
    wcx = nc.dram_tensor("wcx", [128, DIM], F16, kind="ExternalInput").ap()
    xout = nc.dram_tensor("xout", [TPC, DIM], F16, kind="ExternalOutput").ap()
    vout = nc.dram_tensor("vout", [TPC, DIM], F16, kind="ExternalOutput").ap()

    with tile.TileContext(nc) as tc:
        with (
            tc.tile_pool(name="consts", bufs=1) as consts,
            tc.tile_pool(name="tpool", bufs=6) as tpool,
            tc.tile_pool(name="mpool", bufs=4) as mpool,
            tc.tile_pool(name="cpool", bufs=6) as cpool,
            tc.tile_pool(name="spool", bufs=12) as spool,
            tc.tile_pool(name="epool", bufs=1) as epool,
            tc.tile_pool(name="opool", bufs=4) as opool,
            tc.tile_pool(name="ps", bufs=1, space="PSUM") as ps,
        ):
            # ---------------- tiles
            s_bd = consts.tile([128, NV, 128], F16, tag="bd")
            s_wsa = consts.tile([128, 4 * NCH, RANK], F16, tag="wsa")
            s_wcv = consts.tile([128, DIM], F16, tag="wcv")
            s_wcx = consts.tile([128, DIM], F16, tag="wcx")
            s_vt = consts.tile([128, NCH, TPC], F16, tag="vt")
            s_xt = consts.tile([128, NCH, TPC], F16, tag="xt")
            s_vtok = consts.tile([128, NCH, DIM], F16, tag="vtok")
            s_xtok = consts.tile([128, NCH, DIM], F16, tag="xtok")

            B_a = [ps.tile([128, 2 * NC2], F32, tag=f"Ba{c}", name=f"Ba{c}") for c in range(2)]
            B_b = [ps.tile([128, 2 * NC2], F32, tag=f"Bb{c}", name=f"Bb{c}") for c in range(2)]
            B_h = [ps.tile([128, 2 * NC2], F32, tag=f"Bh{c}", name=f"Bh{c}") for c in range(2)]
            B_S = ps.tile([128, NH], F32, tag="BS")
            B_Q = ps.tile([128, NH], F32, tag="BQ")

            asl = slice(0, NC2)  # a/b/h state columns within chain banks
            wsl = slice(NC2, 2 * NC2)  # w columns within B_a

            # memsets: a-banks fully (w region too); b/h state region only
            for c in range(2):
                nc.vector.memset(B_a[c][:], 0.0)
                nc.vector.memset(B_b[c][:, asl], 0.0)
                nc.vector.memset(B_h[c][:, asl], 0.0)
            nc.vector.memset(B_S[:], 0.0)
            nc.vector.memset(B_Q[:], 0.0)

            def bdw(kind, scale):
                return s_bd[:, _vidx(kind, scale), :]

            # ---------------- const + input DMAs (order = DMA device order:
            # wsa first, chain0 pieces, bd, chain1 pieces)
            nc.sync.dma_start(s_wsa[:], wsa[:])
            for ch in range(2):
                c0 = ch * NC2
                for hb in range(2):
                    t0 = hb * NH + c0
                    for src, dst in ((vtr, s_vt), (xtr, s_xt)):
                        sv = src.rearrange("(k p) c -> p k c", p=128)
                        nc.sync.dma_start(
                            dst[:, :, t0 : t0 + NC2], sv[:, :, t0 : t0 + NC2]
                        )
                if ch == 0:
                    nc.sync.dma_start(s_bd[:], bdm[:])
            for ch in range(2):
                c0 = ch * NC2
                for hb in range(2):
                    t0 = hb * NH + c0
                    for k in range(NCH):
                        last = k == NCH - 1
                        # (tsel, moving src, dest bank, dest cols)
                        for tsel, smov, bank, cols in (
                            (2, s_xt, B_h[ch], asl),  # h  (first: gates t1)
                            (1, s_vt, B_b[ch], asl),  # b  (gates m1)
                            (0, s_vt, B_a[ch], asl),  # a
                            (3, s_vt, B_a[ch], wsl),  # w0
                        ):
                            nc.tensor.matmul(
                                bank[hb * 64 : (hb + 1) * 64, cols],
                                s_wsa[:, tsel * NCH + k, :],
                                smov[:, k, t0 : t0 + NC2],
                                start=False,
                                stop=last,
                                tile_position=(0, 64 * hb) if hb else None,
                                skip_group_check=True,
                            )

            # w0 -> fp16 (serves as nw for step 0)
            s_w0 = []
            for ch in range(2):
                w0t = consts.tile([128, NC2], F16, tag=f"w0_{ch}")
                nc.scalar.activation(w0t[:], B_a[ch][:, wsl], COPY)
                s_w0.append(w0t)

            # ---------------- token-major fp32 inputs (needed only at exit)
            nc.sync.dma_start(
                s_vtok[:], vin.rearrange("(tb p) c -> p tb c", p=128)
            )
            nc.sync.dma_start(
                s_xtok[:], xin.rearrange("(tb p) c -> p tb c", p=128)
            )
            nc.sync.dma_start(s_wcv[:], wcv[:])
            nc.sync.dma_start(s_wcx[:], wcx[:])
            # P accumulators (P_n sums on the otherwise-idle Pool engine)
            s_P = []
            for c in range(2):
                pt = consts.tile([128, NC2], F16, tag=f"P{c}", name=f"P{c}")
                nc.gpsimd.memset(pt[:], 0.0)
                s_P.append(pt)

            # ---------------- the 10 RK4 steps
            def mm(bank, sl, kind, scale, rhs, stop=False):
                nc.tensor.matmul(
                    bank[:, sl],
                    bdw(kind, scale),
                    rhs,
                    start=False,
                    stop=stop,
                    skip_group_check=True,
                )

            def step_chain(n, st):
                ch = st["ch"]
                sl = st["sl"]  # chain's columns in B_S/B_Q
                pa, pb, ph = B_a[ch], B_b[ch], B_h[ch]
                last = n == STEPS - 1

                def tanh():
                    t = tpool.tile([128, NC2], F16, tag=f"t{ch}")
                    nc.scalar.activation(t[:], ph[:, asl], TANH)
                    return t

                def prod(b_src, t_s):
                    m = mpool.tile([128, NC2], F16, tag=f"m{ch}")
                    nc.vector.tensor_mul(m[:], b_src, t_s[:])
                    c = cpool.tile([128, NC2], F16, tag=f"c{ch}")
                    nc.vector.tensor_mul(c[:], pa[:, asl], m[:])
                    return c

                # stage 1
                t1 = st.pop("t1n", None)
                if t1 is None:
                    t1 = tanh()
                # h2 = h1 + w,  w_n = w0 - (dt^2/12) Scum_n @ Cax
                mm(ph, asl, "ibd", 1.0, s_w0[ch][:], stop=st["sc_prev"] is None)
                if st["sc_prev"] is not None:
                    mm(ph, asl, "cax", -D2 / 12, st["sc_prev"][:], stop=True)  # h2
                t2 = tanh()
                c1 = prod(pb[:, asl], t1)
                mm(pb, asl, "cab", -DT / 2, c1[:], stop=True)  # b2
                mm(pa, asl, "caa", -DT / 2, c1[:], stop=True)  # a2
                mm(ph, asl, "cax", -D2 / 4, c1[:], stop=True)  # h3
                mm(B_S, sl, "ibd", 1.0, c1[:])
                nc.gpsimd.tensor_add(s_P[ch][:], s_P[ch][:], c1[:])
                yield

                # stage 2
                t3 = tanh()
                c2 = prod(pb[:, asl], t2)
                mm(pb, asl, "cab", DT / 2, c1[:])
                mm(pb, asl, "cab", -DT / 2, c2[:], stop=True)  # b3
                mm(pa, asl, "caa", DT / 2, c1[:])
                mm(pa, asl, "caa", -DT / 2, c2[:], stop=True)  # a3
                mm(ph, asl, "ibd", 1.0, s_w0[ch][:])
                if st["sc_prev"] is not None:
                    mm(ph, asl, "cax", -D2 / 12, st["sc_prev"][:])
                mm(ph, asl, "cax", D2 / 4, c1[:])
                mm(ph, asl, "cax", -D2 / 2, c2[:], stop=True)  # h4
                mm(B_S, sl, "ibd", 2.0, c2[:])
                nc.gpsimd.tensor_add(s_P[ch][:], s_P[ch][:], c2[:])
                yield

                # stage 3; h1' is computable here (h4 is dead once t4 is
                # read), pulling the next step's t1 off the step boundary
                t4 = tanh()
                c3 = prod(pb[:, asl], t3)
                mm(pb, asl, "cab", DT / 2, c2[:])
                mm(pb, asl, "cab", -DT, c3[:], stop=True)  # b4
                if not last:
                    mm(ph, asl, "cax", -D2 / 6, c1[:])
                    mm(ph, asl, "cax", D2 / 3, c2[:])
                    mm(ph, asl, "cax", -D2 / 6, c3[:], stop=True)  # h1'
                    st["t1n"] = tanh()  # tanh(h1') for next step
                mm(pa, asl, "caa", DT / 2, c2[:])
                mm(pa, asl, "caa", -DT, c3[:], stop=True)  # a4
                mm(B_S, sl, "ibd", 2.0, c3[:])
                nc.gpsimd.tensor_add(s_P[ch][:], s_P[ch][:], c3[:])
                yield

                # stage 4
                c4 = prod(pb[:, asl], t4)
                if not last:
                    mm(pb, asl, "cab", -DT / 6, c1[:])
                    mm(pb, asl, "cab", -DT / 3, c2[:])
                    mm(pb, asl, "cab", 2 * DT / 3, c3[:])
                    mm(pb, asl, "cab", -DT / 6, c4[:], stop=True)  # b1'
                    mm(pa, asl, "caa", -DT / 6, c1[:])
                    mm(pa, asl, "caa", -DT / 3, c2[:])
                    mm(pa, asl, "caa", 2 * DT / 3, c3[:])
                    mm(pa, asl, "caa", -DT / 6, c4[:], stop=True)  # a1'
                    mm(B_S, sl, "ibd", 1.0, c4[:])
                    # Scum snapshot: feeds the h-chain w-terms and the exit Q
                    sc = spool.tile([128, NC2], F16, tag=f"sc{ch}")
                    nc.scalar.activation(sc[:], B_S[:, sl], COPY)
                    st["sc_prev"] = sc
                    mm(B_Q, sl, "ibd", 1.0, sc[:])
                    if 2 <= n <= 5:
                        # x0 += v0, one block per chain-step on Pool
                        tb = 2 * (n - 2) + ch
                        nc.gpsimd.tensor_add(
                            s_xtok[:, tb, :], s_xtok[:, tb, :], s_vtok[:, tb, :]
                        )
                else:
                    mm(B_S, sl, "ibd", 1.0, c4[:], stop=(ch == 1))
                yield

            def exit_chain(st):
                ch = st["ch"]
                sl = st["sl"]
                scf = epool.tile([128, NC2], F16, tag=f"scf{ch}")
                nc.scalar.activation(scf[:], B_S[:, sl], COPY)
                mm(B_Q, sl, "ibd", 1.0, s_P[ch][:], stop=(ch == 1))
                qcf = epool.tile([128, NC2], F16, tag=f"qcf{ch}")
                nc.scalar.activation(qcf[:], B_Q[:, sl], COPY)
                banks = [B_a[ch], B_b[ch], B_h[ch]]
                i = 0
                for th in range(2):
                    for tbl in range(2):
                        tb = th * 4 + 2 * ch + tbl
                        ov = opool.tile([128, DIM], F16, tag=f"ov{ch}")
                        ox = opool.tile([128, DIM], F16, tag=f"ox{ch}")
                        for dh in range(2):
                            dsl = slice(dh * NH, (dh + 1) * NH)
                            lhs_S = scf[th * 64 : (th + 1) * 64, tbl * 128 : (tbl + 1) * 128]
                            lhs_Q = qcf[th * 64 : (th + 1) * 64, tbl * 128 : (tbl + 1) * 128]
                            pv = banks[i % 3]
                            px = banks[(i + 1) % 3]
                            i += 2
                            # v half: S-gemm then DVE add of v0
                            nc.tensor.matmul(
                                pv[:],
                                lhs_S,
                                s_wcv[th * 64 : (th + 1) * 64, dsl],
                                start=True,
                                stop=True,
                                tile_position=(64 * th, 0),
                                skip_group_check=True,
                            )
                            nc.vector.tensor_add(ov[:, dsl], pv[:], s_vtok[:, tb, dsl])
                            # x half: (x0+v0) preloaded by identity matmul,
                            # Q-gemm accumulates, ACT materializes
                            nc.tensor.matmul(
                                px[:],
                                bdw("ibd", 1.0),
                                s_xtok[:, tb, dsl],
                                start=True,
                                stop=False,
                                skip_group_check=True,
                            )
                            nc.tensor.matmul(
                                px[:],
                                lhs_Q,
                                s_wcx[th * 64 : (th + 1) * 64, dsl],
                                start=False,
                                stop=True,
                                tile_position=(64 * th, 0),
                                skip_group_check=True,
                            )
                            nc.scalar.activation(ox[:, dsl], px[:], COPY)
                        nc.sync.dma_start(vout[tb * 128 : (tb + 1) * 128, :], ov[:])
                        nc.sync.dma_start(xout[tb * 128 : (tb + 1) * 128, :], ox[:])
                        yield

            chains = [
                {"ch": c, "sl": slice(c * NC2, (c + 1) * NC2), "sc_prev": None}
                for c in range(2)
            ]

            def chain_gen(st):
                for n in range(STEPS):
                    yield from step_chain(n, st)
                yield from exit_chain(st)

            gens = [chain_gen(st) for st in chains]
            # stagger: chain0 six stages ahead so chain0's exit overlaps
            # chain1's last steps and stage-4 PE bursts interleave
            for _ in range(6):
                next(gens[0])
            alive = True
            while alive:
                alive = False
                for g in gens:
                    try:
                        next(g)
                        alive = True
                    except StopIteration:
                        pass

    orig = nc.to_json_bytes
    nc.to_json_bytes = lambda: _split_waits(orig())
    _NC_CACHE = nc
    return nc


# -------------------------------------------------------------------- driver


def _run(x, v, Wa, Wb, Wx, Wc, trace=False):
    from concourse.bass_utils import run_bass_kernel_spmd

    x = np.asarray(x, np.float32).reshape(BATCH * SEQ, DIM)
    v = np.asarray(v, np.float32).reshape(BATCH * SEQ, DIM)
    consts = _host_consts(Wa, Wb, Wx, Wc)

    nc = _build_bass()
    in_maps = []
    for c in range(NCORES):
        xc = np.ascontiguousarray(x[c * TPC : (c + 1) * TPC])
        vc = np.ascontiguousarray(v[c * TPC : (c + 1) * TPC])
        m = {
            "xin": xc.astype(F16NP),
            "vin": vc.astype(F16NP),
            "xt": np.ascontiguousarray(xc.T).astype(F16NP),
            "vt": np.ascontiguousarray(vc.T).astype(F16NP),
        }
        m.update(consts)
        in_maps.append(m)

    res = run_bass_kernel_spmd(
        nc, in_maps, core_ids=list(range(NCORES)), trace=trace
    )
    xo = np.concatenate(
        [np.asarray(res.results[c]["xout"], np.float32) for c in range(NCORES)], axis=0
    )
    vo = np.concatenate(
        [np.asarray(res.results[c]["vout"], np.float32) for c in range(NCORES)], axis=0
    )
    return (xo.reshape(BATCH, SEQ, DIM), vo.reshape(BATCH, SEQ, DIM)), res


def kernel(x, v, Wa, Wb, Wx, Wc):
    (xo, vo), _ = _run(x, v, Wa, Wb, Wx, Wc, trace=False)
    return xo, vo


# revision 13
# speedup vs baseline: 1.4983x; 1.0039x over previous
"""Trainium2 Bass kernel for nn_AdjointManifoldBlock.

Reference computes 10 RK4 steps of:
    dx/dt = v ; dv/dt = -gamma,  gamma = ((v@Wa)*(v@Wb)*tanh(x@Wx)) @ Wc

Rank-space restructuring (per token, rank=64 state):
    a = v@Wa, b = v@Wb, h = x@Wx, w = (dt/2) v@Wx
    c_s = a_s * b_s * tanh(h_s)   per RK4 stage
    every stage update is a [64,64] GEMM with Caa=Wc@Wa, Cab=Wc@Wb, Cax=Wc@Wx
    x_T = x0 + v0 - (dt^2/6) Q @ Wc,  v_T = v0 - (dt/6) S @ Wc
    S = sum S_n, Q = sum [(9-n) S_n + P_n]

Differences vs the earlier version of this kernel:
  - inputs are shipped twice: token-major fp32 (exit adds) and host-transposed
    feature-major fp16 (entry GEMMs) -> no PE transposes, no ACT copies
  - all step-loop matmul operands are fp16 (1 cycle/row at any width)
  - the b-chain lives in PSUM like a (PE undo-accumulation), killing the
    per-stage DVE badd; both per-stage products run on DVE (GPSIMD has no
    PSUM port)
  - the w-chain is derived from the S accumulator: w_n = w0 - (dt^2/12)
    Scum_n @ Cax, tracked in PSUM via per-step delta matmuls from fp16
    snapshots of S (2 matmuls/step instead of 4)
  - exit: x0+v0 pre-added on GPSIMD (idle otherwise) during the steps;
    final adds on DVE straight from PSUM; fp32 x,v DMA'd during the steps

Layout per core (1024 tokens): partition dim = [halfA ranks 0:64 | halfB
ranks 64:128], halves = tokens 0:512 / 512:1024; NSPLIT=2 column chains
(256 cols each) interleaved stage-by-stage for cross-engine overlap.
"""

import json
import numpy as np
import ml_dtypes

DIM = 1024
RANK = 64
STEPS = 10
DT = 0.1
BATCH, SEQ = 4, 2048
NCORES = 8
TPC = (BATCH * SEQ) // NCORES  # tokens per core = 1024
NH = TPC // 2  # tokens per stacked half = 512
NCH = DIM // 128  # feature chunks = 8
NSPLIT = 2
NC2 = NH // NSPLIT  # columns per chain = 256

D2 = DT * DT

CAA_SC = [-DT / 2, DT / 2, -DT, -DT / 6, -DT / 3, 2 * DT / 3]
CAB_SC = list(CAA_SC)
CAX_SC = [-D2 / 4, D2 / 4, -D2 / 2, D2 / 3, -D2 / 6, -D2 / 12, D2 / 12]
IBD_SC = sorted(
    {1.0, 2.0}
    | {float(10 - n) for n in range(STEPS)}
    | {float(19 - 2 * n) for n in range(STEPS)}
    | {float(9 - n) for n in range(STEPS) if 9 - n > 0}
)
NV = len(CAA_SC) + len(CAB_SC) + len(CAX_SC) + len(IBD_SC)

F16NP = np.float16


def _vidx(kind, scale):
    if kind == "caa":
        return CAA_SC.index(scale)
    if kind == "cab":
        return len(CAA_SC) + CAB_SC.index(scale)
    if kind == "cax":
        return len(CAA_SC) + len(CAB_SC) + CAX_SC.index(scale)
    if kind == "ibd":
        return len(CAA_SC) + len(CAB_SC) + len(CAX_SC) + IBD_SC.index(float(scale))
    raise KeyError(kind)


# ---------------------------------------------------------------- host consts


def _host_consts(Wa, Wb, Wx, Wc):
    Wa64 = np.asarray(Wa, np.float64)
    Wb64 = np.asarray(Wb, np.float64)
    Wx64 = np.asarray(Wx, np.float64)
    Wc64 = np.asarray(Wc, np.float64)

    Caa = Wc64 @ Wa64  # [64, 64]; row index = contraction side
    Cab = Wc64 @ Wb64
    Cax = Wc64 @ Wx64
    I64 = np.eye(RANK)

    mats = (
        [Caa * s for s in CAA_SC]
        + [Cab * s for s in CAB_SC]
        + [Cax * s for s in CAX_SC]
        + [I64 * s for s in IBD_SC]
    )
    bd = np.zeros((NV, 128, 128), np.float64)
    for i, m in enumerate(mats):
        bd[i, 0:64, 0:64] = m
        bd[i, 64:128, 64:128] = m
    bd = np.ascontiguousarray(bd.transpose(1, 0, 2)).astype(F16NP)  # [128, NV, 128]

    stk = np.stack(
        [W.reshape(NCH, 128, RANK) for W in (Wa64, Wb64, Wx64, (DT / 2) * Wx64)]
    )  # [4, 8, 128, 64]
    wsa = np.ascontiguousarray(stk.transpose(2, 0, 1, 3).reshape(128, 4 * NCH, RANK)).astype(
        F16NP
    )
    wcv1 = -(DT / 6) * Wc64  # [64, 1024]
    wcx1 = -(D2 / 6) * Wc64
    wcv = np.concatenate([wcv1, wcv1], axis=0).astype(F16NP)  # [128, 1024] duplicated
    wcx = np.concatenate([wcx1, wcx1], axis=0).astype(F16NP)

    return {"bd": bd, "wsa": wsa, "wcv": wcv, "wcx": wcx}


# ----------------------------------------------------------- BIR wait postpass


def _split_waits(data: bytes) -> bytes:
    """This walrus build accepts only one inline sync wait per instruction;
    move excess waits onto NoOps inserted before the instruction (the
    engine sequencer processes them in order, so semantics are identical)."""
    bir = json.loads(data)
    for fn in bir["functions"]:
        for blk in fn["blocks"]:
            out = []
            k = 0
            for inst in blk["instructions"]:
                si = inst.get("sync_info")
                if si and len(si.get("on_wait", [])) > 1:
                    waits = si["on_wait"]
                    pre = []
                    while len(waits) > 1:
                        chunk, waits = waits[:1], waits[1:]
                        k += 1
                        pre.append(
                            {
                                "name": f"{inst['name']}-w{k}",
                                "opcode": "NoOp",
                                "engine": inst["engine"],
                                "ins": [],
                                "outs": [],
                                "sync_info": {"on_wait": chunk, "on_update": []},
                            }
                        )
                    si["on_wait"] = waits
                    out.extend(pre)
                out.append(inst)
            blk["instructions"] = out
    return json.dumps(bir).encode()


# ---------------------------------------------------------------- bass builder

_NC_CACHE = None


def _build_bass():
    global _NC_CACHE
    if _NC_CACHE is not None:
        return _NC_CACHE

    import concourse.bass as bass
    import concourse.tile as tile
    import concourse.mybir as mybir

    F32 = mybir.dt.float32
    F16 = mybir.dt.float16
    TANH = mybir.ActivationFunctionType.Tanh
    COPY = mybir.ActivationFunctionType.Copy

    nc = bass.Bass("TRN2", target_bir_lowering=False, debug=False, num_devices=1)

    xin = nc.dram_tensor("xin", [TPC, DIM], F16, kind="ExternalInput").ap()
    vin = nc.dram_tensor("vin", [TPC, DIM], F16, kind="ExternalInput").ap()
    xtr = nc.dram_tensor("xt", [DIM, TPC], F16, kind="ExternalInput").ap()
    vtr = nc.dram_tensor("vt", [DIM, TPC], F16, kind="ExternalInput").ap()
    bdm = nc.dram_tensor("bd", [128, NV, 128], F16, kind="ExternalInput").ap()
    wsa = nc.dram_tensor("wsa", [128, 4 * NCH, RANK], F16, kind="ExternalInput").ap()
    wcv = nc.dram_tensor("wcv", [128, DIM], F16, kind="ExternalInput").ap()
    wcx = nc.dram_tensor("wcx", [128, DIM], F16, kind="ExternalInput").ap()
    xout = nc.dram_tensor("xout", [TPC, DIM], F16, kind="ExternalOutput").ap()
    vout = nc.dram_tensor("vout", [TPC, DIM], F16, kind="ExternalOutput").ap()

    with tile.TileContext(nc) as tc:
        with (
            tc.tile_pool(name="consts", bufs=1) as consts,
            tc.tile_pool(name="tpool", bufs=6) as tpool,
            tc.tile_pool(name="mpool", bufs=6) as mpool,
            tc.tile_pool(name="cpool", bufs=10) as cpool,
            tc.tile_pool(name="spool", bufs=12) as spool,
            tc.tile_pool(name="epool", bufs=1) as epool,
            tc.tile_pool(name="opool", bufs=4) as opool,
            tc.tile_pool(name="ps", bufs=1, space="PSUM") as ps,
        ):
            # ---------------- tiles
            s_bd = consts.tile([128, NV, 128], F16, tag="bd")
            s_wsa = consts.tile([128, 4 * NCH, RANK], F16, tag="wsa")
            s_wcv = consts.tile([128, DIM], F16, tag="wcv")
            s_wcx = consts.tile([128, DIM], F16, tag="wcx")
            s_vt = consts.tile([128, NCH, TPC], F16, tag="vt")
            s_xt = consts.tile([128, NCH, TPC], F16, tag="xt")
            s_vtok = consts.tile([128, NCH, DIM], F16, tag="vtok")
            s_xtok = consts.tile([128, NCH, DIM], F16, tag="xtok")

            B_a = [ps.tile([128, 2 * NC2], F32, tag=f"Ba{c}", name=f"Ba{c}") for c in range(2)]
            B_b = [ps.tile([128, 2 * NC2], F32, tag=f"Bb{c}", name=f"Bb{c}") for c in range(2)]
            B_h = [ps.tile([128, 2 * NC2], F32, tag=f"Bh{c}", name=f"Bh{c}") for c in range(2)]
            B_S = ps.tile([128, NH], F32, tag="BS")
            B_Q = ps.tile([128, NH], F32, tag="BQ")

            asl = slice(0, NC2)  # a/b/h state columns within chain banks
            wsl = slice(NC2, 2 * NC2)  # w columns within B_a

            # memsets: a-banks fully (w region too); b/h state region only
            for c in range(2):
                nc.vector.memset(B_a[c][:], 0.0)
                nc.vector.memset(B_b[c][:, asl], 0.0)
                nc.vector.memset(B_h[c][:, asl], 0.0)
            nc.vector.memset(B_S[:], 0.0)
            nc.vector.memset(B_Q[:], 0.0)

            def bdw(kind, scale):
                return s_bd[:, _vidx(kind, scale), :]

            # ---------------- const + input DMAs (order = DMA device order:
            # wsa first, chain0 pieces, bd, chain1 pieces)
            nc.sync.dma_start(s_wsa[:], wsa[:])
            for ch in range(2):
                c0 = ch * NC2
                for hb in range(2):
                    t0 = hb * NH + c0
                    for src, dst in ((vtr, s_vt), (xtr, s_xt)):
                        sv = src.rearrange("(k p) c -> p k c", p=128)
                        nc.sync.dma_start(
                            dst[:, :, t0 : t0 + NC2], sv[:, :, t0 : t0 + NC2]
                        )
                if ch == 0:
                    nc.sync.dma_start(s_bd[:], bdm[:])
            for ch in range(2):
                c0 = ch * NC2
                for hb in range(2):
                    t0 = hb * NH + c0
                    for k in range(NCH):
                        last = k == NCH - 1
                        # (tsel, moving src, dest bank, dest cols)
                        for tsel, smov, bank, cols in (
                            (2, s_xt, B_h[ch], asl),  # h  (first: gates t1)
                            (1, s_vt, B_b[ch], asl),  # b  (gates m1)
                            (0, s_vt, B_a[ch], asl),  # a
                            (3, s_vt, B_a[ch], wsl),  # w0
                        ):
                            nc.tensor.matmul(
                                bank[hb * 64 : (hb + 1) * 64, cols],
                                s_wsa[:, tsel * NCH + k, :],
                                smov[:, k, t0 : t0 + NC2],
                                start=False,
                                stop=last,
                                tile_position=(0, 64 * hb) if hb else None,
                                skip_group_check=True,
                            )

            # w0 -> fp16 (serves as nw for step 0)
            s_w0 = []
            for ch in range(2):
                w0t = consts.tile([128, NC2], F16, tag=f"w0_{ch}")
                nc.scalar.activation(w0t[:], B_a[ch][:, wsl], COPY)
                s_w0.append(w0t)

            # ---------------- token-major fp32 inputs (needed only at exit)
            nc.sync.dma_start(
                s_vtok[:], vin.rearrange("(tb p) c -> p tb c", p=128)
            )
            nc.sync.dma_start(
                s_xtok[:], xin.rearrange("(tb p) c -> p tb c", p=128)
            )
            nc.sync.dma_start(s_wcv[:], wcv[:])
            nc.sync.dma_start(s_wcx[:], wcx[:])
            # P accumulators (P_n sums on the otherwise-idle Pool engine)
            s_P = []
            for c in range(2):
                pt = consts.tile([128, NC2], F16, tag=f"P{c}", name=f"P{c}")
                nc.gpsimd.memset(pt[:], 0.0)
                s_P.append(pt)

            # ---------------- the 10 RK4 steps
            def mm(bank, sl, kind, scale, rhs, stop=False):
                nc.tensor.matmul(
                    bank[:, sl],
                    bdw(kind, scale),
                    rhs,
                    start=False,
                    stop=stop,
                    skip_group_check=True,
                )

            def step_chain(n, st):
                ch = st["ch"]
                sl = st["sl"]  # chain's columns in B_S/B_Q
                pa, pb, ph = B_a[ch], B_b[ch], B_h[ch]
                last = n == STEPS - 1

                def tanh():
                    t = tpool.tile([128, NC2], F16, tag=f"t{ch}")
                    nc.scalar.activation(t[:], ph[:, asl], TANH)
                    return t

                def prod(b_src, t_s):
                    m = mpool.tile([128, NC2], F16, tag=f"m{ch}")
                    nc.vector.tensor_mul(m[:], b_src, t_s[:])
                    c = cpool.tile([128, NC2], F16, tag=f"c{ch}")
                    nc.vector.tensor_mul(c[:], pa[:, asl], m[:])
                    return c

                # stage 1
                t1 = st.pop("t1n", None)
                if t1 is None:
                    t1 = tanh()
                # h2 = h1 + w,  w_n = w0 - (dt^2/12) Scum_n @ Cax
                mm(ph, asl, "ibd", 1.0, s_w0[ch][:], stop=st["sc_prev"] is None)
                if st["sc_prev"] is not None:
                    mm(ph, asl, "cax", -D2 / 12, st["sc_prev"][:], stop=True)  # h2
                t2 = tanh()
                c1 = prod(pb[:, asl], t1)
                mm(pb, asl, "cab", -DT / 2, c1[:], stop=True)  # b2
                mm(pa, asl, "caa", -DT / 2, c1[:], stop=True)  # a2
                mm(ph, asl, "cax", -D2 / 4, c1[:], stop=True)  # h3
                mm(B_S, sl, "ibd", 1.0, c1[:])
                nc.gpsimd.tensor_add(s_P[ch][:], s_P[ch][:], c1[:])
                yield

                # stage 2
                t3 = tanh()
                c2 = prod(pb[:, asl], t2)
                mm(pb, asl, "cab", DT / 2, c1[:])
                mm(pb, asl, "cab", -DT / 2, c2[:], stop=True)  # b3
                mm(pa, asl, "caa", DT / 2, c1[:])
                mm(pa, asl, "caa", -DT / 2, c2[:], stop=True)  # a3
                mm(ph, asl, "ibd", 1.0, s_w0[ch][:])
                if st["sc_prev"] is not None:
                    mm(ph, asl, "cax", -D2 / 12, st["sc_prev"][:])
                mm(ph, asl, "cax", D2 / 4, c1[:])
                mm(ph, asl, "cax", -D2 / 2, c2[:], stop=True)  # h4
                mm(B_S, sl, "ibd", 2.0, c2[:])
                nc.gpsimd.tensor_add(s_P[ch][:], s_P[ch][:], c2[:])
                yield

                # stage 3; h1' is computable here (h4 is dead once t4 is
                # read), pulling the next step's t1 off the step boundary
                t4 = tanh()
                c3 = prod(pb[:, asl], t3)
                mm(pb, asl, "cab", DT / 2, c2[:])
                mm(pb, asl, "cab", -DT, c3[:], stop=True)  # b4
                if not last:
                    mm(ph, asl, "cax", -D2 / 6, c1[:])
                    mm(ph, asl, "cax", D2 / 3, c2[:])
                    mm(ph, asl, "cax", -D2 / 6, c3[:], stop=True)  # h1'
                    st["t1n"] = tanh()  # tanh(h1') for next step
                mm(pa, asl, "caa", DT / 2, c2[:])
                mm(pa, asl, "caa", -DT, c3[:], stop=True)  # a4
                mm(B_S, sl, "ibd", 2.0, c3[:])
                nc.gpsimd.tensor_add(s_P[ch][:], s_P[ch][:], c3[:])
                yield

                # stage 4
                c4 = prod(pb[:, asl], t4)
                if not last:
                    mm(pb, asl, "cab", -DT / 6, c1[:])
                    mm(pb, asl, "cab", -DT / 3, c2[:])
                    mm(pb, asl, "cab", 2 * DT / 3, c3[:])
                    mm(pb, asl, "cab", -DT / 6, c4[:], stop=True)  # b1'
                    mm(pa, asl, "caa", -DT / 6, c1[:])
                    mm(pa, asl, "caa", -DT / 3, c2[:])
                    mm(pa, asl, "caa", 2 * DT / 3, c3[:])
                    mm(pa, asl, "caa", -DT / 6, c4[:], stop=True)  # a1'
                    mm(B_S, sl, "ibd", 1.0, c4[:])
                    # Scum snapshot: feeds the h-chain w-terms and the exit Q
                    sc = spool.tile([128, NC2], F16, tag=f"sc{ch}")
                    nc.scalar.activation(sc[:], B_S[:, sl], COPY)
                    st["sc_prev"] = sc
                    mm(B_Q, sl, "ibd", 1.0, sc[:])
                    if 1 <= n <= 4:
                        # x0 += v0, one half-block per chain-stage-4 on Pool
                        for half in range(2):
                            s8 = 4 * (n - 1) + 2 * ch + half
                            tb, dh = s8 // 2, s8 % 2
                            hs = slice(dh * NH, (dh + 1) * NH)
                            nc.gpsimd.tensor_add(
                                s_xtok[:, tb, hs], s_xtok[:, tb, hs], s_vtok[:, tb, hs]
                            )
                else:
                    mm(B_S, sl, "ibd", 1.0, c4[:], stop=(ch == 1))
                yield

            def exit_chain(st):
                ch = st["ch"]
                sl = st["sl"]
                scf = epool.tile([128, NC2], F16, tag=f"scf{ch}")
                nc.scalar.activation(scf[:], B_S[:, sl], COPY)
                mm(B_Q, sl, "ibd", 1.0, s_P[ch][:], stop=(ch == 1))
                qcf = epool.tile([128, NC2], F16, tag=f"qcf{ch}")
                nc.scalar.activation(qcf[:], B_Q[:, sl], COPY)
                banks = [B_a[ch], B_b[ch], B_h[ch]]
                i = 0
                for th in range(2):
                    for tbl in range(2):
                        tb = th * 4 + 2 * ch + tbl
                        ov = opool.tile([128, DIM], F16, tag=f"ov{ch}")
                        ox = opool.tile([128, DIM], F16, tag=f"ox{ch}")
                        for dh in range(2):
                            dsl = slice(dh * NH, (dh + 1) * NH)
                            lhs_S = scf[th * 64 : (th + 1) * 64, tbl * 128 : (tbl + 1) * 128]
                            lhs_Q = qcf[th * 64 : (th + 1) * 64, tbl * 128 : (tbl + 1) * 128]
                            pv = banks[i % 3]
                            px = banks[(i + 1) % 3]
                            i += 2
                            # v half: S-gemm then DVE add of v0
                            nc.tensor.matmul(
                                pv[:],
                                lhs_S,
                                s_wcv[th * 64 : (th + 1) * 64, dsl],
                                start=True,
                                stop=True,
                                tile_position=(64 * th, 0),
                                skip_group_check=True,
                            )
                            nc.vector.tensor_add(ov[:, dsl], pv[:], s_vtok[:, tb, dsl])
                            # x half: (x0+v0) preloaded by identity matmul,
                            # Q-gemm accumulates, ACT materializes
                            nc.tensor.matmul(
                                px[:],
                                bdw("ibd", 1.0),
                                s_xtok[:, tb, dsl],
                                start=True,
                                stop=False,
                                skip_group_check=True,
                            )
                            nc.tensor.matmul(
                                px[:],
                                lhs_Q,
                                s_wcx[th * 64 : (th + 1) * 64, dsl],
                                start=False,
                                stop=True,
                                tile_position=(64 * th, 0),
                                skip_group_check=True,
                            )
                            nc.scalar.activation(ox[:, dsl], px[:], COPY)
                        nc.sync.dma_start(vout[tb * 128 : (tb + 1) * 128, :], ov[:])
                        nc.sync.dma_start(xout[tb * 128 : (tb + 1) * 128, :], ox[:])
                        yield

            chains = [
                {"ch": c, "sl": slice(c * NC2, (c + 1) * NC2), "sc_prev": None}
                for c in range(2)
            ]

            def chain_gen(st):
                for n in range(STEPS):
                    yield from step_chain(n, st)
                yield from exit_chain(st)

            gens = [chain_gen(st) for st in chains]
            # stagger: chain0 six stages ahead so chain0's exit overlaps
            # chain1's last steps and stage-4 PE bursts interleave
            for _ in range(6):
                next(gens[0])
            alive = True
            while alive:
                alive = False
                for g in gens:
                    try:
                        next(g)
                        alive = True
                    except StopIteration:
                        pass

    orig = nc.to_json_bytes
    nc.to_json_bytes = lambda: _split_waits(orig())
    _NC_CACHE = nc
    return nc


# -------------------------------------------------------------------- driver


def _run(x, v, Wa, Wb, Wx, Wc, trace=False):
    from concourse.bass_utils import run_bass_kernel_spmd

    x = np.asarray(x, np.float32).reshape(BATCH * SEQ, DIM)
    v = np.asarray(v, np.float32).reshape(BATCH * SEQ, DIM)
    consts = _host_consts(Wa, Wb, Wx, Wc)

    nc = _build_bass()
    in_maps = []
    for c in range(NCORES):
        xc = np.ascontiguousarray(x[c * TPC : (c + 1) * TPC])
        vc = np.ascontiguousarray(v[c * TPC : (c + 1) * TPC])
        m = {
            "xin": xc.astype(F16NP),
            "vin": vc.astype(F16NP),
            "xt": np.ascontiguousarray(xc.T).astype(F16NP),
            "vt": np.ascontiguousarray(vc.T).astype(F16NP),
        }
        m.update(consts)
        in_maps.append(m)

    res = run_bass_kernel_spmd(
        nc, in_maps, core_ids=list(range(NCORES)), trace=trace
    )
    xo = np.concatenate(
        [np.asarray(res.results[c]["xout"], np.float32) for c in range(NCORES)], axis=0
    )
    vo = np.concatenate(
        [np.asarray(res.results[c]["vout"], np.float32) for c in range(NCORES)], axis=0
    )
    return (xo.reshape(BATCH, SEQ, DIM), vo.reshape(BATCH, SEQ, DIM)), res


def kernel(x, v, Wa, Wb, Wx, Wc):
    (xo, vo), _ = _run(x, v, Wa, Wb, Wx, Wc, trace=False)
    return xo, vo
